# revision 15
# baseline (speedup 1.0000x reference)
"""DeepFM forward kernel for Trainium2, data-parallel over 8 NeuronCores.

Model (B=4096, N=16, D=256):
  gram[b]   = x[b] @ x[b].T                       (second-order interactions)
  h1        = relu(flat(x) @ W1 + b1)             (4096 -> 1024)
  h2        = relu(h1 @ W2 + b2)                  (1024 -> 512)
  higher    = h2 @ W3 + b3                        (512 -> 64)
  out       = [higher, triu(gram)] @ Wc + bc      (184 -> 5)

Sharding: batch split 8 ways (512 rows/core), all weights replicated.

On-chip layout is feature-on-partition ("transposed activations"):
activations live as [feat, batch] so every layer is a natural
weight-stationary matmul  hT_L = W_L.T @ hT_{L-1}  with contraction on the
partition dim.  x is transposed host-side so no on-chip transpose is needed.

The per-sample gram matrices are computed by packing 8 samples into one
128x128 matmul (columns = (sample, field)); the 8 useful 16x16 diagonal
blocks are pulled out through a DRAM scratch roundtrip with a strided
gather, directly into [pair, batch] layout.  The upper-triangle selection
is folded into the classifier weights host-side (rows for i>=j get zeros),
so the final layer contracts over all 256 (i,j) cells plus the 64 deep
features in one accumulated matmul chain.
"""

import os
import numpy as np
from contextlib import ExitStack

import concourse.bass as bass
import concourse.bacc as bacc
import concourse.mybir as mybir
import concourse.tile as tile
from concourse.bass_utils import run_bass_kernel_spmd
from concourse.masks import make_identity

# Problem shape (hardcoded per contest rules).
B, N, D = 4096, 16, 256
ND = N * D              # 4096
H1, H2, HO = 1024, 512, 64
PAIRS = N * (N - 1) // 2
C = 5
NCORES = 8
BL = B // NCORES        # 512 rows per core

P = 128
F32 = mybir.dt.float32
F32R = mybir.dt.float32r
BF16 = mybir.dt.bfloat16

AFT = mybir.ActivationFunctionType

# dtype knobs (bitcast views for matmul inputs)
MLP_MM_DT = F32    # dense MLP matmul input dtype

K1 = ND // P        # 32 k-tiles for layer 1
OT1 = H1 // P       # 8 output tiles layer 1
K2 = H1 // P        # 8
OT2 = H2 // P       # 4
K3 = H2 // P        # 4
NGROUP = BL // 8    # 64 gram groups of 8 samples
GCHUNK = 8          # gram groups per extraction chunk
NCHUNK = NGROUP // GCHUNK  # 8


def _mm_view(ap, dt):
    return ap.bitcast(dt) if dt != ap.dtype else ap


def _build_program():
    nc = bacc.Bacc(
        "TRN2",
        target_bir_lowering=False,
        debug=False,
        num_devices=NCORES,
    )

    # x shard, host-rearranged: xtg[t, dlocal, b, i] = x[b, i, t*128+dlocal]
    # -> gram operands are contiguous [128,128] slices; layer-1 rhs for
    # k-tile (i,t) is the stride-16 column set  [:, t, :, i]
    xt_d = nc.dram_tensor("xtg", [2, P, BL, N], F32, kind="ExternalInput")
    # w1r: host-rearranged [OT1, ND, 128] so each output-tile slab is contiguous
    w1_d = nc.dram_tensor("w1r", [OT1, ND, P], F32, kind="ExternalInput")
    w2_d = nc.dram_tensor("w2r", [OT2, H1, P], F32, kind="ExternalInput")
    w3_d = nc.dram_tensor("w3", [H2, HO], F32, kind="ExternalInput")
    b1_d = nc.dram_tensor("b1", [H1], F32, kind="ExternalInput")
    b2_d = nc.dram_tensor("b2", [H2], F32, kind="ExternalInput")
    b3_d = nc.dram_tensor("b3", [HO], F32, kind="ExternalInput")
    # classifier, host-padded: wca [128,5] (64 deep rows + zeros),
    # wcb/wcc [128,5] for gram cells i in 0..7 / 8..15 (zeros where i>=j)
    wca_d = nc.dram_tensor("wca", [P, C], F32, kind="ExternalInput")
    wcb_d = nc.dram_tensor("wcb", [P, C], F32, kind="ExternalInput")
    wcc_d = nc.dram_tensor("wcc", [P, C], F32, kind="ExternalInput")
    bc_d = nc.dram_tensor("bc", [C], F32, kind="ExternalInput")
    out_d = nc.dram_tensor("out", [C, BL], F32, kind="ExternalOutput")

    with tile.TileContext(nc) as tc:
        with ExitStack() as ctx:
            _kernel_body(
                ctx, tc,
                xt_d, w1_d, w2_d, w3_d, b1_d, b2_d, b3_d,
                wca_d, wcb_d, wcc_d, bc_d, out_d,
            )
    nc.compile()
    return nc


def _kernel_body(ctx, tc, xt_d, w1_d, w2_d, w3_d, b1_d, b2_d, b3_d,
                 wca_d, wcb_d, wcc_d, bc_d, out_d):
    nc = tc.nc

    pool_const = ctx.enter_context(tc.tile_pool(name="const", bufs=1))
    pool_xt = ctx.enter_context(tc.tile_pool(name="xt", bufs=1))
    pool_w1 = ctx.enter_context(tc.tile_pool(name="w1", bufs=2))
    pool_h1 = ctx.enter_context(tc.tile_pool(name="h1", bufs=1))
    pool_h2 = ctx.enter_context(tc.tile_pool(name="h2", bufs=1))
    pool_gram = ctx.enter_context(tc.tile_pool(name="gram", bufs=2))
    pool_gramT = ctx.enter_context(tc.tile_pool(name="gramT", bufs=1))
    pool_psum = ctx.enter_context(tc.tile_pool(name="psum", bufs=2, space="PSUM"))
    pool_psum_g = ctx.enter_context(
        tc.tile_pool(name="psum_g", bufs=2, space="PSUM"))
    pool_dram = ctx.enter_context(tc.tile_pool(name="scratch", bufs=1, space="DRAM"))

    # ---- constants / weights ----
    b1_sb = pool_const.tile([P, OT1], F32)
    nc.scalar.dma_start(b1_sb[:], b1_d.ap().rearrange("(o p) -> p o", p=P))
    b2_sb = pool_const.tile([P, OT2], F32)
    nc.scalar.dma_start(b2_sb[:], b2_d.ap().rearrange("(o p) -> p o", p=P))
    b3_sb = pool_const.tile([HO, 1], F32)
    nc.scalar.dma_start(b3_sb[:], b3_d.ap().rearrange("(p o) -> p o", o=1))
    bc_sb = pool_const.tile([C, 1], F32)
    nc.scalar.dma_start(bc_sb[:], bc_d.ap().rearrange("(p o) -> p o", o=1))
    wca_sb = pool_const.tile([P, C], F32)
    nc.scalar.dma_start(wca_sb[:], wca_d.ap())
    wcb_sb = pool_const.tile([P, C], F32)
    nc.scalar.dma_start(wcb_sb[:], wcb_d.ap())
    wcc_sb = pool_const.tile([P, C], F32)
    nc.scalar.dma_start(wcc_sb[:], wcc_d.ap())

    # W2 slabs: [H1, 128] per output tile, resident
    w2_sb = pool_const.tile([P, OT2, K2, P], F32)
    for o2 in range(OT2):
        nc.scalar.dma_start(
            w2_sb[:, o2],
            w2_d.ap()[o2].rearrange("(k p) o -> p k o", p=P),
        )
    # W3: [H2, 64] resident
    w3_sb = pool_const.tile([P, K3, HO], F32)
    nc.scalar.dma_start(w3_sb[:], w3_d.ap().rearrange("(k p) o -> p k o", p=P))

    # ---- x^T (rearranged): [128, t, b, i] ----
    xtg_sb = pool_xt.tile([P, 2, BL, N], F32)
    for t in range(2):
        for bc in range(4):
            nc.sync.dma_start(
                xtg_sb[:, t, bc * 128:(bc + 1) * 128],
                xt_d.ap()[t, :, bc * 128:(bc + 1) * 128],
            )

    def l1_rhs(kt):
        # k-tile kt = 2*i + t covers features f = i*256 + t*128 + dlocal
        i, t = divmod(kt, 2)
        return bass.AP(
            xtg_sb.tensor,
            xtg_sb.offset + t * BL * N + i,
            [[2 * BL * N, P], [N, BL]],
        )

    # ---- layer-1 slab loader ----
    def load_w1_slab(ot):
        slab = pool_w1.tile([P, K1, P], F32, tag="w1slab")
        for cch in range(4):
            k0 = cch * 8
            nc.sync.dma_start(
                slab[:, k0:k0 + 8],
                w1_d.ap()[ot, k0 * P:(k0 + 8) * P].rearrange(
                    "(k p) o -> p k o", p=P),
            )
        return slab

    h1_sb = pool_h1.tile([P, K2, BL], F32)

    def l1_pass(ot, slab):
        ps = pool_psum.tile([P, BL], F32, tag="ps_mlp")
        for kt in range(K1):
            nc.tensor.matmul(
                ps[:],
                _mm_view(slab[:, kt], MLP_MM_DT),
                _mm_view(l1_rhs(kt), MLP_MM_DT),
                start=(kt == 0),
                stop=(kt == K1 - 1),
            )
        nc.scalar.activation(
            h1_sb[:, ot], ps[:], AFT.Relu, bias=b1_sb[:, ot:ot + 1])

    # ---- layer 1, first pass (overlaps with x^T arrival) ----
    slab0 = load_w1_slab(0)
    l1_pass(0, slab0)

    # ---- gram: 8 samples per 128x128 fp32 matmul ----
    def gram_operand(g, t):
        # columns m = s*16 + i are contiguous in xtg layout
        return xtg_sb[:, t, g * 8:(g + 1) * 8, :]

    scratch = pool_dram.tile([NGROUP * P * P], F32)

    gram_chunks = []
    for ch in range(NCHUNK):
        gbuf = pool_gram.tile([P, GCHUNK, P], F32, tag="gbuf")
        for gl in range(GCHUNK):
            g = ch * GCHUNK + gl
            pg = pool_psum_g.tile([P, P], F32, tag="ps_gram")
            for t in range(2):
                nc.tensor.matmul(
                    pg[:],
                    gram_operand(g, t),
                    gram_operand(g, t),
                    start=(t == 0),
                    stop=(t == 1),
                )
            nc.vector.tensor_copy(gbuf[:, gl], pg[:])
        # scatter chunk to DRAM scratch: addr(g,p,f) = (g*128+p)*128+f
        nc.scalar.dma_start(
            bass.AP(scratch.tensor,
                    scratch.offset + ch * GCHUNK * P * P,
                    [[P, P], [P * P, GCHUNK], [1, P]]),
            gbuf[:],
        )
        gram_chunks.append(gbuf)

    # gather diagonal 16x16 blocks into gramT halves [128, BL]
    # dst partition = (i%8)*16 + j, free = g*8 + s
    # src elem addr = g*16384 + s*2064 + i*128 + j  (within chunk)
    # gather diagonal 16x16 blocks into natural layout G_sb[b, (i,j)]:
    # one DMA per group, dims (s, i, j), b-contiguous on both sides
    g_sb = pool_gramT.tile([P, 4, N * N], F32, tag="g_nat", name="g_nat")
    for g in range(NGROUP):
        src = bass.AP(
            scratch.tensor,
            scratch.offset + g * P * P,
            [[2064, 8], [P, N], [1, N]],
        )
        dst = bass.AP(
            g_sb.tensor,
            g_sb.offset + (g % 16) * 8 * (4 * N * N) + (g // 16) * N * N,
            [[4 * N * N, 8], [N, N], [1, N]],
        )
        nc.sync.dma_start(dst, src)

    # PE-transpose 128x128 blocks of G_sb into gramT[h] = [cells, b]
    identity = pool_const.tile([P, P], F32)
    make_identity(nc, identity)
    gramT = [
        pool_gramT.tile([P, BL], F32, tag=f"gramT{h}", name=f"gramT{h}")
        for h in range(2)
    ]
    for bt in range(4):
        for h in range(2):
            pt = pool_psum_g.tile([P, P], F32, tag="ps_tr", name="ps_tr")
            nc.tensor.transpose(
                pt[:], g_sb[:, bt, h * P:(h + 1) * P], identity[:])
            nc.vector.tensor_copy(gramT[h][:, bt * P:(bt + 1) * P], pt[:])

    # ---- layer 1, remaining passes ----
    for ot in range(1, OT1):
        slab = load_w1_slab(ot)
        l1_pass(ot, slab)

    # ---- layer 2 ----
    h2_sb = pool_h2.tile([P, K3, BL], F32)
    for o2 in range(OT2):
        ps = pool_psum.tile([P, BL], F32, tag="ps_mlp")
        for kt in range(K2):
            nc.tensor.matmul(
                ps[:],
                _mm_view(w2_sb[:, o2, kt], MLP_MM_DT),
                _mm_view(h1_sb[:, kt], MLP_MM_DT),
                start=(kt == 0),
                stop=(kt == K2 - 1),
            )
        nc.scalar.activation(
            h2_sb[:, o2], ps[:], AFT.Relu, bias=b2_sb[:, o2:o2 + 1])

    # ---- layer 3 -> higher [64, BL] padded to 128 rows of zeros ----
    higher_sb = pool_gramT.tile([P, BL], F32, tag="higher")
    nc.gpsimd.memset(higher_sb[HO:, :], 0.0)
    ps3 = pool_psum.tile([P, BL], F32, tag="ps_mlp")
    for kt in range(K3):
        nc.tensor.matmul(
            ps3[:HO],
            _mm_view(w3_sb[:, kt], MLP_MM_DT),
            _mm_view(h2_sb[:, kt], MLP_MM_DT),
            start=(kt == 0),
            stop=(kt == K3 - 1),
        )
    nc.scalar.activation(
        higher_sb[:HO], ps3[:HO], AFT.Identity, bias=b3_sb[:])

    # ---- classifier: out[5, BL] = Wc.T @ [higher; gram cells] + bc ----
    psf = pool_psum.tile([P, BL], F32, tag="ps_mlp")
    nc.tensor.matmul(psf[:C], wca_sb[:], higher_sb[:], start=True, stop=False)
    nc.tensor.matmul(psf[:C], wcb_sb[:], gramT[0][:], start=False, stop=False)
    nc.tensor.matmul(psf[:C], wcc_sb[:], gramT[1][:], start=False, stop=True)
    out_sb = pool_const.tile([C, BL], F32)
    nc.scalar.activation(out_sb[:], psf[:C], AFT.Identity, bias=bc_sb[:])
    nc.sync.dma_start(out_d.ap(), out_sb[:])


_CACHED = None


def _get_program():
    global _CACHED
    if _CACHED is None:
        _CACHED = _build_program()
    return _CACHED


def prepare_in_maps(x, W1, b1, W2, b2, W3, b3, Wc, bc):
    x = np.ascontiguousarray(np.asarray(x, dtype=np.float32))
    W1 = np.asarray(W1, dtype=np.float32)
    W2 = np.asarray(W2, dtype=np.float32)
    W3 = np.ascontiguousarray(np.asarray(W3, dtype=np.float32))
    b1 = np.asarray(b1, dtype=np.float32)
    b2 = np.asarray(b2, dtype=np.float32)
    b3 = np.asarray(b3, dtype=np.float32)
    Wc = np.asarray(Wc, dtype=np.float32)
    bc = np.asarray(bc, dtype=np.float32)

    # host-side layout prep (replicated operands)
    w1r = np.ascontiguousarray(
        W1.reshape(ND, OT1, P).transpose(1, 0, 2))   # [8, 4096, 128]
    w2r = np.ascontiguousarray(
        W2.reshape(H1, OT2, P).transpose(1, 0, 2))   # [4, 1024, 128]

    # classifier padding: fold triu-pair selection into gram-cell weights
    iu, ju = np.triu_indices(N, k=1)
    wc_gram = np.zeros((N * N, C), dtype=np.float32)
    wc_gram[iu * N + ju] = Wc[HO:]
    wca = np.zeros((P, C), dtype=np.float32)
    wca[:HO] = Wc[:HO]
    wcb = np.ascontiguousarray(wc_gram[:P])
    wcc = np.ascontiguousarray(wc_gram[P:])

    common = dict(
        w1r=w1r, w2r=w2r, w3=W3, b1=b1, b2=b2, b3=b3,
        wca=wca, wcb=wcb, wcc=wcc, bc=bc,
    )
    return [
        dict(common, xtg=np.ascontiguousarray(
            x[c * BL:(c + 1) * BL]
            .reshape(BL, N, 2, P).transpose(2, 3, 0, 1)))
        for c in range(NCORES)
    ]


def kernel(x, W1, b1, W2, b2, W3, b3, Wc, bc):
    in_maps = prepare_in_maps(x, W1, b1, W2, b2, W3, b3, Wc, bc)
    nc = _get_program()
    res = run_bass_kernel_spmd(nc, in_maps, core_ids=list(range(NCORES)))
    out = np.empty((B, C), dtype=np.float32)
    for c in range(NCORES):
        out[c * BL:(c + 1) * BL] = res.results[c]["out"].T
    return out


# revision 16
# speedup vs baseline: 1.5958x; 1.5958x over previous
"""DeepFM forward kernel for Trainium2, data-parallel over 8 NeuronCores.

Model (B=4096, N=16, D=256):
  gram[b]   = x[b] @ x[b].T                       (second-order interactions)
  h1        = relu(flat(x) @ W1 + b1)             (4096 -> 1024)
  h2        = relu(h1 @ W2 + b2)                  (1024 -> 512)
  higher    = h2 @ W3 + b3                        (512 -> 64)
  out       = [higher, triu(gram)] @ Wc + bc      (184 -> 5)

Sharding: batch split 8 ways (512 rows/core), all weights replicated.

On-chip layout is feature-on-partition ("transposed activations"):
activations live as [feat, batch] so every layer is a natural
weight-stationary matmul  hT_L = W_L.T @ hT_{L-1}  with contraction on the
partition dim.  x is transposed host-side so no on-chip transpose is needed.

The per-sample gram matrices are computed by packing 8 samples into one
128x128 matmul (columns = (sample, field)); the 8 useful 16x16 diagonal
blocks are pulled out through a DRAM scratch roundtrip with a strided
gather, directly into [pair, batch] layout.  The upper-triangle selection
is folded into the classifier weights host-side (rows for i>=j get zeros),
so the final layer contracts over all 256 (i,j) cells plus the 64 deep
features in one accumulated matmul chain.
"""

import os
import numpy as np
from contextlib import ExitStack

import concourse.bass as bass
import concourse.bacc as bacc
import concourse.mybir as mybir
import concourse.tile as tile
from concourse.bass_utils import run_bass_kernel_spmd
from concourse.masks import make_identity

# Problem shape (hardcoded per contest rules).
B, N, D = 4096, 16, 256
ND = N * D              # 4096
H1, H2, HO = 1024, 512, 64
PAIRS = N * (N - 1) // 2
C = 5
NCORES = 8
BL = B // NCORES        # 512 rows per core

P = 128
F32 = mybir.dt.float32
F32R = mybir.dt.float32r
BF16 = mybir.dt.bfloat16

AFT = mybir.ActivationFunctionType

# dtype knob for the dense MLP matmuls (layers 1-3): bf16 halves weight
# DMA traffic and runs the PE at 1 cycle/row (fp32 runs at 4).
MLP_BF16 = True
MLP_DT = BF16 if MLP_BF16 else F32

K1 = ND // P        # 32 k-tiles for layer 1
OT1 = H1 // P       # 8 output tiles layer 1
K2 = H1 // P        # 8
OT2 = H2 // P       # 4
K3 = H2 // P        # 4
NGROUP = BL // 8    # 64 gram groups of 8 samples
GCHUNK = 8          # gram groups per extraction chunk
NCHUNK = NGROUP // GCHUNK  # 8


def _mm_view(ap, dt):
    return ap.bitcast(dt) if dt != ap.dtype else ap


def _build_program():
    nc = bacc.Bacc(
        "TRN2",
        target_bir_lowering=False,
        debug=False,
        num_devices=NCORES,
    )

    # x shard, host-rearranged: xtg[t, dlocal, b, i] = x[b, i, t*128+dlocal]
    # -> gram operands are contiguous [128,128] slices; layer-1 rhs for
    # k-tile (i,t) is the stride-16 column set  [:, t, :, i]
    xt_d = nc.dram_tensor("xtg", [2, P, BL, N], F32, kind="ExternalInput")
    # w1r: host-rearranged [OT1, ND, 128] so each output-tile slab is contiguous
    w1_d = nc.dram_tensor("w1r", [OT1, ND, P], MLP_DT, kind="ExternalInput")
    w2_d = nc.dram_tensor("w2r", [OT2, H1, P], MLP_DT, kind="ExternalInput")
    w3_d = nc.dram_tensor("w3", [H2, HO], MLP_DT, kind="ExternalInput")
    b1_d = nc.dram_tensor("b1", [H1], F32, kind="ExternalInput")
    b2_d = nc.dram_tensor("b2", [H2], F32, kind="ExternalInput")
    b3_d = nc.dram_tensor("b3", [HO], F32, kind="ExternalInput")
    # classifier, host-padded: wca [128,5] (64 deep rows + zeros),
    # wcb/wcc [128,5] for gram cells i in 0..7 / 8..15 (zeros where i>=j)
    wca_d = nc.dram_tensor("wca", [P, C], F32, kind="ExternalInput")
    wcb_d = nc.dram_tensor("wcb", [P, C], F32, kind="ExternalInput")
    wcc_d = nc.dram_tensor("wcc", [P, C], F32, kind="ExternalInput")
    bc_d = nc.dram_tensor("bc", [C], F32, kind="ExternalInput")
    out_d = nc.dram_tensor("out", [C, BL], F32, kind="ExternalOutput")

    with tile.TileContext(nc) as tc:
        with ExitStack() as ctx:
            _kernel_body(
                ctx, tc,
                xt_d, w1_d, w2_d, w3_d, b1_d, b2_d, b3_d,
                wca_d, wcb_d, wcc_d, bc_d, out_d,
            )
    nc.compile()
    return nc


def _kernel_body(ctx, tc, xt_d, w1_d, w2_d, w3_d, b1_d, b2_d, b3_d,
                 wca_d, wcb_d, wcc_d, bc_d, out_d):
    nc = tc.nc

    pool_const = ctx.enter_context(tc.tile_pool(name="const", bufs=1))
    pool_xt = ctx.enter_context(tc.tile_pool(name="xt", bufs=1))
    pool_w1 = ctx.enter_context(tc.tile_pool(name="w1", bufs=2))
    pool_h1 = ctx.enter_context(tc.tile_pool(name="h1", bufs=1))
    pool_h2 = ctx.enter_context(tc.tile_pool(name="h2", bufs=1))
    pool_gram = ctx.enter_context(tc.tile_pool(name="gram", bufs=2))
    pool_gramT = ctx.enter_context(tc.tile_pool(name="gramT", bufs=1))
    pool_psum = ctx.enter_context(tc.tile_pool(name="psum", bufs=2, space="PSUM"))
    pool_psum_g = ctx.enter_context(
        tc.tile_pool(name="psum_g", bufs=2, space="PSUM"))
    pool_dram = ctx.enter_context(tc.tile_pool(name="scratch", bufs=1, space="DRAM"))

    # ---- constants / weights ----
    b1_sb = pool_const.tile([P, OT1], F32)
    nc.scalar.dma_start(b1_sb[:], b1_d.ap().rearrange("(o p) -> p o", p=P))
    b2_sb = pool_const.tile([P, OT2], F32)
    nc.scalar.dma_start(b2_sb[:], b2_d.ap().rearrange("(o p) -> p o", p=P))
    b3_sb = pool_const.tile([HO, 1], F32)
    nc.scalar.dma_start(b3_sb[:], b3_d.ap().rearrange("(p o) -> p o", o=1))
    bc_sb = pool_const.tile([C, 1], F32)
    nc.scalar.dma_start(bc_sb[:], bc_d.ap().rearrange("(p o) -> p o", o=1))
    wca_sb = pool_const.tile([P, C], F32)
    nc.scalar.dma_start(wca_sb[:], wca_d.ap())
    wcb_sb = pool_const.tile([P, C], F32)
    nc.scalar.dma_start(wcb_sb[:], wcb_d.ap())
    wcc_sb = pool_const.tile([P, C], F32)
    nc.scalar.dma_start(wcc_sb[:], wcc_d.ap())

    # W2 slabs: [H1, 128] per output tile, resident
    w2_sb = pool_const.tile([P, OT2, K2, P], MLP_DT)
    for o2 in range(OT2):
        nc.scalar.dma_start(
            w2_sb[:, o2],
            w2_d.ap()[o2].rearrange("(k p) o -> p k o", p=P),
        )
    # W3: [H2, 64] resident
    w3_sb = pool_const.tile([P, K3, HO], MLP_DT)
    nc.scalar.dma_start(w3_sb[:], w3_d.ap().rearrange("(k p) o -> p k o", p=P))

    # ---- x^T (rearranged): [128, t, b, i] ----
    xtg_sb = pool_xt.tile([P, 2, BL, N], F32)
    for t in range(2):
        for bc in range(4):
            nc.sync.dma_start(
                xtg_sb[:, t, bc * 128:(bc + 1) * 128],
                xt_d.ap()[t, :, bc * 128:(bc + 1) * 128],
            )

    if MLP_BF16:
        xl1_sb = pool_xt.tile([P, 2, BL, N], MLP_DT)
        for t in range(2):
            for bc in range(4):
                nc.vector.tensor_copy(
                    xl1_sb[:, t, bc * 128:(bc + 1) * 128],
                    xtg_sb[:, t, bc * 128:(bc + 1) * 128])
    else:
        xl1_sb = xtg_sb

    def l1_rhs(kt):
        # k-tile kt = 2*i + t covers features f = i*256 + t*128 + dlocal
        i, t = divmod(kt, 2)
        return bass.AP(
            xl1_sb.tensor,
            xl1_sb.offset + t * BL * N + i,
            [[2 * BL * N, P], [N, BL]],
        )

    # ---- layer-1 slab loader ----
    def load_w1_slab(ot):
        slab = pool_w1.tile([P, K1, P], MLP_DT, tag="w1slab")
        for cch in range(4):
            k0 = cch * 8
            nc.sync.dma_start(
                slab[:, k0:k0 + 8],
                w1_d.ap()[ot, k0 * P:(k0 + 8) * P].rearrange(
                    "(k p) o -> p k o", p=P),
            )
        return slab

    h1_sb = pool_h1.tile([P, K2, BL], MLP_DT)

    def l1_pass(ot, slab):
        ps = pool_psum.tile([P, BL], F32, tag="ps_mlp")
        for kt in range(K1):
            nc.tensor.matmul(
                ps[:],
                slab[:, kt],
                l1_rhs(kt),
                start=(kt == 0),
                stop=(kt == K1 - 1),
            )
        nc.scalar.activation(
            h1_sb[:, ot], ps[:], AFT.Relu, bias=b1_sb[:, ot:ot + 1])

    # ---- layer 1, first pass (overlaps with x^T arrival) ----
    slab0 = load_w1_slab(0)
    l1_pass(0, slab0)

    # ---- gram: 8 samples per 128x128 fp32 matmul ----
    def gram_operand(g, t):
        # columns m = s*16 + i are contiguous in xtg layout
        return xtg_sb[:, t, g * 8:(g + 1) * 8, :]

    scratch = pool_dram.tile([NGROUP * P * P], F32)

    gram_chunks = []
    for ch in range(NCHUNK):
        gbuf = pool_gram.tile([P, GCHUNK, P], F32, tag="gbuf")
        for gl in range(GCHUNK):
            g = ch * GCHUNK + gl
            pg = pool_psum_g.tile([P, P], F32, tag="ps_gram")
            for t in range(2):
                nc.tensor.matmul(
                    pg[:],
                    gram_operand(g, t),
                    gram_operand(g, t),
                    start=(t == 0),
                    stop=(t == 1),
                )
            nc.vector.tensor_copy(gbuf[:, gl], pg[:])
        # scatter chunk to DRAM scratch: addr(g,p,f) = (g*128+p)*128+f
        nc.scalar.dma_start(
            bass.AP(scratch.tensor,
                    scratch.offset + ch * GCHUNK * P * P,
                    [[P, P], [P * P, GCHUNK], [1, P]]),
            gbuf[:],
        )
        gram_chunks.append(gbuf)

    # gather diagonal 16x16 blocks into gramT halves [128, BL]
    # dst partition = (i%8)*16 + j, free = g*8 + s
    # src elem addr = g*16384 + s*2064 + i*128 + j  (within chunk)
    # gather diagonal 16x16 blocks into natural layout G_sb[b, (i,j)]:
    # one DMA per group, dims (s, i, j), b-contiguous on both sides
    g_sb = pool_gramT.tile([P, 4, N * N], F32, tag="g_nat", name="g_nat")
    for g in range(NGROUP):
        src = bass.AP(
            scratch.tensor,
            scratch.offset + g * P * P,
            [[2064, 8], [P, N], [1, N]],
        )
        dst = bass.AP(
            g_sb.tensor,
            g_sb.offset + (g % 16) * 8 * (4 * N * N) + (g // 16) * N * N,
            [[4 * N * N, 8], [N, N], [1, N]],
        )
        nc.sync.dma_start(dst, src)

    # PE-transpose 128x128 blocks of G_sb into gramT[h] = [cells, b]
    identity = pool_const.tile([P, P], F32)
    make_identity(nc, identity)
    gramT = [
        pool_gramT.tile([P, BL], F32, tag=f"gramT{h}", name=f"gramT{h}")
        for h in range(2)
    ]
    for bt in range(4):
        for h in range(2):
            pt = pool_psum_g.tile([P, P], F32, tag="ps_tr", name="ps_tr")
            nc.tensor.transpose(
                pt[:], g_sb[:, bt, h * P:(h + 1) * P], identity[:])
            nc.vector.tensor_copy(gramT[h][:, bt * P:(bt + 1) * P], pt[:])

    # ---- layer 1, remaining passes ----
    for ot in range(1, OT1):
        slab = load_w1_slab(ot)
        l1_pass(ot, slab)

    # ---- layer 2 ----
    h2_sb = pool_h2.tile([P, K3, BL], MLP_DT)
    for o2 in range(OT2):
        ps = pool_psum.tile([P, BL], F32, tag="ps_mlp")
        for kt in range(K2):
            nc.tensor.matmul(
                ps[:],
                w2_sb[:, o2, kt],
                h1_sb[:, kt],
                start=(kt == 0),
                stop=(kt == K2 - 1),
            )
        nc.scalar.activation(
            h2_sb[:, o2], ps[:], AFT.Relu, bias=b2_sb[:, o2:o2 + 1])

    # ---- layer 3 -> higher [64, BL] padded to 128 rows of zeros ----
    higher_sb = pool_gramT.tile([P, BL], F32, tag="higher")
    nc.gpsimd.memset(higher_sb[HO:, :], 0.0)
    ps3 = pool_psum.tile([P, BL], F32, tag="ps_mlp")
    for kt in range(K3):
        nc.tensor.matmul(
            ps3[:HO],
            w3_sb[:, kt],
            h2_sb[:, kt],
            start=(kt == 0),
            stop=(kt == K3 - 1),
        )
    nc.scalar.activation(
        higher_sb[:HO], ps3[:HO], AFT.Identity, bias=b3_sb[:])

    # ---- classifier: out[5, BL] = Wc.T @ [higher; gram cells] + bc ----
    psf = pool_psum.tile([P, BL], F32, tag="ps_mlp")
    nc.tensor.matmul(psf[:C], wca_sb[:], higher_sb[:], start=True, stop=False)
    nc.tensor.matmul(psf[:C], wcb_sb[:], gramT[0][:], start=False, stop=False)
    nc.tensor.matmul(psf[:C], wcc_sb[:], gramT[1][:], start=False, stop=True)
    out_sb = pool_const.tile([C, BL], F32)
    nc.scalar.activation(out_sb[:], psf[:C], AFT.Identity, bias=bc_sb[:])
    nc.sync.dma_start(out_d.ap(), out_sb[:])


_CACHED = None


def _get_program():
    global _CACHED
    if _CACHED is None:
        _CACHED = _build_program()
    return _CACHED


def prepare_in_maps(x, W1, b1, W2, b2, W3, b3, Wc, bc):
    x = np.ascontiguousarray(np.asarray(x, dtype=np.float32))
    W1 = np.asarray(W1, dtype=np.float32)
    W2 = np.asarray(W2, dtype=np.float32)
    W3 = np.ascontiguousarray(np.asarray(W3, dtype=np.float32))
    b1 = np.asarray(b1, dtype=np.float32)
    b2 = np.asarray(b2, dtype=np.float32)
    b3 = np.asarray(b3, dtype=np.float32)
    Wc = np.asarray(Wc, dtype=np.float32)
    bc = np.asarray(bc, dtype=np.float32)

    # host-side layout prep (replicated operands)
    import ml_dtypes
    wdt = ml_dtypes.bfloat16 if MLP_BF16 else np.float32
    w1r = np.ascontiguousarray(
        W1.reshape(ND, OT1, P).transpose(1, 0, 2).astype(wdt))
    w2r = np.ascontiguousarray(
        W2.reshape(H1, OT2, P).transpose(1, 0, 2).astype(wdt))
    W3 = W3.astype(wdt)

    # classifier padding: fold triu-pair selection into gram-cell weights
    iu, ju = np.triu_indices(N, k=1)
    wc_gram = np.zeros((N * N, C), dtype=np.float32)
    wc_gram[iu * N + ju] = Wc[HO:]
    wca = np.zeros((P, C), dtype=np.float32)
    wca[:HO] = Wc[:HO]
    wcb = np.ascontiguousarray(wc_gram[:P])
    wcc = np.ascontiguousarray(wc_gram[P:])

    common = dict(
        w1r=w1r, w2r=w2r, w3=W3, b1=b1, b2=b2, b3=b3,
        wca=wca, wcb=wcb, wcc=wcc, bc=bc,
    )
    return [
        dict(common, xtg=np.ascontiguousarray(
            x[c * BL:(c + 1) * BL]
            .reshape(BL, N, 2, P).transpose(2, 3, 0, 1)))
        for c in range(NCORES)
    ]


def kernel(x, W1, b1, W2, b2, W3, b3, Wc, bc):
    in_maps = prepare_in_maps(x, W1, b1, W2, b2, W3, b3, Wc, bc)
    nc = _get_program()
    res = run_bass_kernel_spmd(nc, in_maps, core_ids=list(range(NCORES)))
    out = np.empty((B, C), dtype=np.float32)
    for c in range(NCORES):
        out[c * BL:(c + 1) * BL] = res.results[c]["out"].T
    return out


# revision 17
# speedup vs baseline: 3.2343x; 2.0267x over previous
"""DeepFM forward kernel for Trainium2, data-parallel over 8 NeuronCores.

Model (B=4096, N=16, D=256):
  gram[b]   = x[b] @ x[b].T                       (second-order interactions)
  h1        = relu(flat(x) @ W1 + b1)             (4096 -> 1024)
  h2        = relu(h1 @ W2 + b2)                  (1024 -> 512)
  higher    = h2 @ W3 + b3                        (512 -> 64)
  out       = [higher, triu(gram)] @ Wc + bc      (184 -> 5)

Sharding: batch split 8 ways (512 rows/core), all weights replicated.

On-chip layout is feature-on-partition ("transposed activations"):
activations live as [feat, batch] so every layer is a natural
weight-stationary matmul  hT_L = W_L.T @ hT_{L-1}  with contraction on the
partition dim.  x is transposed host-side so no on-chip transpose is needed.

The per-sample gram matrices are computed by packing 8 samples into one
128x128 matmul (columns = (sample, field)); the 8 useful 16x16 diagonal
blocks are pulled out through a DRAM scratch roundtrip with a strided
gather, directly into [pair, batch] layout.  The upper-triangle selection
is folded into the classifier weights host-side (rows for i>=j get zeros),
so the final layer contracts over all 256 (i,j) cells plus the 64 deep
features in one accumulated matmul chain.
"""

import os
import numpy as np
from contextlib import ExitStack

import concourse.bass as bass
import concourse.bacc as bacc
import concourse.mybir as mybir
import concourse.tile as tile
from concourse.bass_utils import run_bass_kernel_spmd
from concourse.masks import make_identity

# Problem shape (hardcoded per contest rules).
B, N, D = 4096, 16, 256
ND = N * D              # 4096
H1, H2, HO = 1024, 512, 64
PAIRS = N * (N - 1) // 2
C = 5
NCORES = 8
BL = B // NCORES        # 512 rows per core

P = 128
F32 = mybir.dt.float32
F32R = mybir.dt.float32r
BF16 = mybir.dt.bfloat16

AFT = mybir.ActivationFunctionType

# dtype knob for the dense MLP matmuls (layers 1-3): bf16 halves weight
# DMA traffic and runs the PE at 1 cycle/row (fp32 runs at 4).
MLP_BF16 = True
MLP_DT = BF16 if MLP_BF16 else F32

K1 = ND // P        # 32 k-tiles for layer 1
OT1 = H1 // P       # 8 output tiles layer 1
K2 = H1 // P        # 8
OT2 = H2 // P       # 4
K3 = H2 // P        # 4
NGROUP = BL // 8    # 64 gram groups of 8 samples
GCHUNK = 8          # gram groups per extraction chunk
NCHUNK = NGROUP // GCHUNK  # 8


def _mm_view(ap, dt):
    return ap.bitcast(dt) if dt != ap.dtype else ap


def _build_program():
    nc = bacc.Bacc(
        "TRN2",
        target_bir_lowering=False,
        debug=False,
        num_devices=NCORES,
    )

    # x shard in two bf16 layouts:
    #   xg[t, dlocal, b, i] = x[b, i, t*128+dlocal]  (gram: contiguous
    #     [128,128] operands per 8-sample group)
    #   xl1[kt, dlocal, b] = k-tile-major x^T  (layer-1: contiguous rhs)
    xg_d = nc.dram_tensor("xg", [2, P, BL, N], BF16, kind="ExternalInput")
    xl1_d = nc.dram_tensor("xl1", [K1, P, BL], BF16, kind="ExternalInput")
    # w1r: host-rearranged [OT1, ND, 128] so each output-tile slab is contiguous
    w1_d = nc.dram_tensor("w1r", [OT1, ND, P], MLP_DT, kind="ExternalInput")
    w2_d = nc.dram_tensor("w2r", [OT2, H1, P], MLP_DT, kind="ExternalInput")
    w3_d = nc.dram_tensor("w3", [H2, HO], MLP_DT, kind="ExternalInput")
    b1_d = nc.dram_tensor("b1", [H1], F32, kind="ExternalInput")
    b2_d = nc.dram_tensor("b2", [H2], F32, kind="ExternalInput")
    b3_d = nc.dram_tensor("b3", [HO], F32, kind="ExternalInput")
    # classifier, host-padded: wca [128,5] (64 deep rows + zeros),
    # wcb/wcc [128,5] for gram cells i in 0..7 / 8..15 (zeros where i>=j)
    wca_d = nc.dram_tensor("wca", [P, C], F32, kind="ExternalInput")
    wcb_d = nc.dram_tensor("wcb", [P, C], F32, kind="ExternalInput")
    wcc_d = nc.dram_tensor("wcc", [P, C], F32, kind="ExternalInput")
    bc_d = nc.dram_tensor("bc", [C], F32, kind="ExternalInput")
    out_d = nc.dram_tensor("out", [C, BL], F32, kind="ExternalOutput")

    with tile.TileContext(nc) as tc:
        with ExitStack() as ctx:
            _kernel_body(
                ctx, tc,
                xg_d, xl1_d, w1_d, w2_d, w3_d, b1_d, b2_d, b3_d,
                wca_d, wcb_d, wcc_d, bc_d, out_d,
            )
    nc.compile()
    return nc


def _kernel_body(ctx, tc, xg_d, xl1_d, w1_d, w2_d, w3_d, b1_d, b2_d, b3_d,
                 wca_d, wcb_d, wcc_d, bc_d, out_d):
    nc = tc.nc

    pool_const = ctx.enter_context(tc.tile_pool(name="const", bufs=1))
    pool_xt = ctx.enter_context(tc.tile_pool(name="xt", bufs=1))
    pool_w1 = ctx.enter_context(tc.tile_pool(name="w1", bufs=2))
    pool_h1 = ctx.enter_context(tc.tile_pool(name="h1", bufs=1))
    pool_h2 = ctx.enter_context(tc.tile_pool(name="h2", bufs=1))
    pool_gram = ctx.enter_context(tc.tile_pool(name="gram", bufs=2))
    pool_gramT = ctx.enter_context(tc.tile_pool(name="gramT", bufs=1))
    pool_psum = ctx.enter_context(tc.tile_pool(name="psum", bufs=2, space="PSUM"))
    pool_psum_g = ctx.enter_context(
        tc.tile_pool(name="psum_g", bufs=2, space="PSUM"))
    pool_dram = ctx.enter_context(tc.tile_pool(name="scratch", bufs=1, space="DRAM"))

    # ---- constants / weights ----
    b1_sb = pool_const.tile([P, OT1], F32)
    nc.scalar.dma_start(b1_sb[:], b1_d.ap().rearrange("(o p) -> p o", p=P))
    b2_sb = pool_const.tile([P, OT2], F32)
    nc.scalar.dma_start(b2_sb[:], b2_d.ap().rearrange("(o p) -> p o", p=P))
    b3_sb = pool_const.tile([HO, 1], F32)
    nc.scalar.dma_start(b3_sb[:], b3_d.ap().rearrange("(p o) -> p o", o=1))
    bc_sb = pool_const.tile([C, 1], F32)
    nc.scalar.dma_start(bc_sb[:], bc_d.ap().rearrange("(p o) -> p o", o=1))
    wca_sb = pool_const.tile([P, C], F32)
    nc.scalar.dma_start(wca_sb[:], wca_d.ap())
    wcb_sb = pool_const.tile([P, C], F32)
    nc.scalar.dma_start(wcb_sb[:], wcb_d.ap())
    wcc_sb = pool_const.tile([P, C], F32)
    nc.scalar.dma_start(wcc_sb[:], wcc_d.ap())

    # W2 slabs: [H1, 128] per output tile, resident
    w2_sb = pool_const.tile([P, OT2, K2, P], MLP_DT)
    for o2 in range(OT2):
        nc.scalar.dma_start(
            w2_sb[:, o2],
            w2_d.ap()[o2].rearrange("(k p) o -> p k o", p=P),
        )
    # W3: [H2, 64] resident
    w3_sb = pool_const.tile([P, K3, HO], MLP_DT)
    nc.scalar.dma_start(w3_sb[:], w3_d.ap().rearrange("(k p) o -> p k o", p=P))

    # ---- x (both layouts) ----
    xg_sb = pool_xt.tile([P, 2, BL, N], BF16)
    for t in range(2):
        for bc4 in range(2):
            nc.sync.dma_start(
                xg_sb[:, t, bc4 * 256:(bc4 + 1) * 256],
                xg_d.ap()[t, :, bc4 * 256:(bc4 + 1) * 256],
            )
    xl1_sb = pool_xt.tile([P, K1, BL], BF16)
    for k4 in range(8):
        nc.sync.dma_start(
            xl1_sb[:, k4 * 4:(k4 + 1) * 4],
            xl1_d.ap()[k4 * 4:(k4 + 1) * 4].rearrange("k p b -> p k b"),
        )

    def l1_rhs(kt):
        return xl1_sb[:, kt]

    # ---- layer-1 slab loader ----
    def load_w1_slab(ot):
        slab = pool_w1.tile([P, K1, P], MLP_DT, tag="w1slab")
        for cch in range(4):
            k0 = cch * 8
            nc.sync.dma_start(
                slab[:, k0:k0 + 8],
                w1_d.ap()[ot, k0 * P:(k0 + 8) * P].rearrange(
                    "(k p) o -> p k o", p=P),
            )
        return slab

    h1_sb = pool_h1.tile([P, K2, BL], MLP_DT)

    def l1_pass(ot, slab):
        ps = pool_psum.tile([P, BL], F32, tag="ps_mlp")
        for kt in range(K1):
            nc.tensor.matmul(
                ps[:],
                slab[:, kt],
                l1_rhs(kt),
                start=(kt == 0),
                stop=(kt == K1 - 1),
            )
        nc.scalar.activation(
            h1_sb[:, ot], ps[:], AFT.Relu, bias=b1_sb[:, ot:ot + 1])

    # ---- layer 1, first pass (overlaps with x^T arrival) ----
    slab0 = load_w1_slab(0)
    l1_pass(0, slab0)

    # ---- gram: 8 samples per 128x128 bf16 matmul ----
    def gram_operand(g, t):
        # columns m = s*16 + i are contiguous in the xg layout
        return xg_sb[:, t, g * 8:(g + 1) * 8, :]

    scratch = pool_dram.tile([NGROUP * P * P], F32)

    gram_chunks = []
    for ch in range(NCHUNK):
        gbuf = pool_gram.tile([P, GCHUNK, P], F32, tag="gbuf")
        for gl in range(GCHUNK):
            g = ch * GCHUNK + gl
            pg = pool_psum_g.tile([P, P], F32, tag="ps_gram")
            for t in range(2):
                nc.tensor.matmul(
                    pg[:],
                    gram_operand(g, t),
                    gram_operand(g, t),
                    start=(t == 0),
                    stop=(t == 1),
                )
            nc.vector.tensor_copy(gbuf[:, gl], pg[:])
        # scatter chunk to DRAM scratch: addr(g,p,f) = (g*128+p)*128+f
        nc.scalar.dma_start(
            bass.AP(scratch.tensor,
                    scratch.offset + ch * GCHUNK * P * P,
                    [[P, P], [P * P, GCHUNK], [1, P]]),
            gbuf[:],
        )
        gram_chunks.append(gbuf)

    # gather diagonal 16x16 blocks into gramT halves [128, BL]
    # dst partition = (i%8)*16 + j, free = g*8 + s
    # src elem addr = g*16384 + s*2064 + i*128 + j  (within chunk)
    # gather diagonal 16x16 blocks into natural layout G_sb[b, (i,j)]:
    # one DMA per group, dims (s, i, j), b-contiguous on both sides
    g_sb = pool_gramT.tile([P, 4, N * N], F32, tag="g_nat", name="g_nat")
    for g in range(NGROUP):
        src = bass.AP(
            scratch.tensor,
            scratch.offset + g * P * P,
            [[2064, 8], [P, N], [1, N]],
        )
        dst = bass.AP(
            g_sb.tensor,
            g_sb.offset + (g % 16) * 8 * (4 * N * N) + (g // 16) * N * N,
            [[4 * N * N, 8], [N, N], [1, N]],
        )
        nc.sync.dma_start(dst, src)

    # PE-transpose 128x128 blocks of G_sb into gramT[h] = [cells, b]
    identity = pool_const.tile([P, P], F32)
    make_identity(nc, identity)
    gramT = [
        pool_gramT.tile([P, BL], F32, tag=f"gramT{h}", name=f"gramT{h}")
        for h in range(2)
    ]
    for bt in range(4):
        for h in range(2):
            pt = pool_psum_g.tile([P, P], F32, tag="ps_tr", name="ps_tr")
            nc.tensor.transpose(
                pt[:], g_sb[:, bt, h * P:(h + 1) * P], identity[:])
            nc.vector.tensor_copy(gramT[h][:, bt * P:(bt + 1) * P], pt[:])

    # ---- layer 1, remaining passes ----
    for ot in range(1, OT1):
        slab = load_w1_slab(ot)
        l1_pass(ot, slab)

    # ---- layer 2 ----
    h2_sb = pool_h2.tile([P, K3, BL], MLP_DT)
    for o2 in range(OT2):
        ps = pool_psum.tile([P, BL], F32, tag="ps_mlp")
        for kt in range(K2):
            nc.tensor.matmul(
                ps[:],
                w2_sb[:, o2, kt],
                h1_sb[:, kt],
                start=(kt == 0),
                stop=(kt == K2 - 1),
            )
        nc.scalar.activation(
            h2_sb[:, o2], ps[:], AFT.Relu, bias=b2_sb[:, o2:o2 + 1])

    # ---- layer 3 -> higher [64, BL] padded to 128 rows of zeros ----
    higher_sb = pool_gramT.tile([P, BL], F32, tag="higher")
    nc.gpsimd.memset(higher_sb[HO:, :], 0.0)
    ps3 = pool_psum.tile([P, BL], F32, tag="ps_mlp")
    for kt in range(K3):
        nc.tensor.matmul(
            ps3[:HO],
            w3_sb[:, kt],
            h2_sb[:, kt],
            start=(kt == 0),
            stop=(kt == K3 - 1),
        )
    nc.scalar.activation(
        higher_sb[:HO], ps3[:HO], AFT.Identity, bias=b3_sb[:])

    # ---- classifier: out[5, BL] = Wc.T @ [higher; gram cells] + bc ----
    psf = pool_psum.tile([P, BL], F32, tag="ps_mlp")
    nc.tensor.matmul(psf[:C], wca_sb[:], higher_sb[:], start=True, stop=False)
    nc.tensor.matmul(psf[:C], wcb_sb[:], gramT[0][:], start=False, stop=False)
    nc.tensor.matmul(psf[:C], wcc_sb[:], gramT[1][:], start=False, stop=True)
    out_sb = pool_const.tile([C, BL], F32)
    nc.scalar.activation(out_sb[:], psf[:C], AFT.Identity, bias=bc_sb[:])
    nc.sync.dma_start(out_d.ap(), out_sb[:])


_CACHED = None


def _get_program():
    global _CACHED
    if _CACHED is None:
        _CACHED = _build_program()
    return _CACHED


def prepare_in_maps(x, W1, b1, W2, b2, W3, b3, Wc, bc):
    x = np.ascontiguousarray(np.asarray(x, dtype=np.float32))
    W1 = np.asarray(W1, dtype=np.float32)
    W2 = np.asarray(W2, dtype=np.float32)
    W3 = np.ascontiguousarray(np.asarray(W3, dtype=np.float32))
    b1 = np.asarray(b1, dtype=np.float32)
    b2 = np.asarray(b2, dtype=np.float32)
    b3 = np.asarray(b3, dtype=np.float32)
    Wc = np.asarray(Wc, dtype=np.float32)
    bc = np.asarray(bc, dtype=np.float32)

    # host-side layout prep (replicated operands)
    import ml_dtypes
    wdt = ml_dtypes.bfloat16 if MLP_BF16 else np.float32
    w1r = np.ascontiguousarray(
        W1.reshape(ND, OT1, P).transpose(1, 0, 2).astype(wdt))
    w2r = np.ascontiguousarray(
        W2.reshape(H1, OT2, P).transpose(1, 0, 2).astype(wdt))
    W3 = W3.astype(wdt)

    # classifier padding: fold triu-pair selection into gram-cell weights
    iu, ju = np.triu_indices(N, k=1)
    wc_gram = np.zeros((N * N, C), dtype=np.float32)
    wc_gram[iu * N + ju] = Wc[HO:]
    wca = np.zeros((P, C), dtype=np.float32)
    wca[:HO] = Wc[:HO]
    wcb = np.ascontiguousarray(wc_gram[:P])
    wcc = np.ascontiguousarray(wc_gram[P:])

    common = dict(
        w1r=w1r, w2r=w2r, w3=W3, b1=b1, b2=b2, b3=b3,
        wca=wca, wcb=wcb, wcc=wcc, bc=bc,
    )
    bf = ml_dtypes.bfloat16
    maps = []
    for c in range(NCORES):
        xs = x[c * BL:(c + 1) * BL].reshape(BL, N, 2, P).astype(bf)
        maps.append(dict(
            common,
            xg=np.ascontiguousarray(xs.transpose(2, 3, 0, 1)),
            xl1=np.ascontiguousarray(
                xs.transpose(1, 2, 3, 0).reshape(K1, P, BL)),
        ))
    return maps


def kernel(x, W1, b1, W2, b2, W3, b3, Wc, bc):
    in_maps = prepare_in_maps(x, W1, b1, W2, b2, W3, b3, Wc, bc)
    nc = _get_program()
    res = run_bass_kernel_spmd(nc, in_maps, core_ids=list(range(NCORES)))
    out = np.empty((B, C), dtype=np.float32)
    for c in range(NCORES):
        out[c * BL:(c + 1) * BL] = res.results[c]["out"].T
    return out


# revision 18
# speedup vs baseline: 3.4062x; 1.0532x over previous
"""DeepFM forward kernel for Trainium2, data-parallel over 8 NeuronCores.

Model (B=4096, N=16, D=256):
  gram[b]   = x[b] @ x[b].T                       (second-order interactions)
  h1        = relu(flat(x) @ W1 + b1)             (4096 -> 1024)
  h2        = relu(h1 @ W2 + b2)                  (1024 -> 512)
  higher    = h2 @ W3 + b3                        (512 -> 64)
  out       = [higher, triu(gram)] @ Wc + bc      (184 -> 5)

Sharding: batch split 8 ways (512 rows/core), all weights replicated.

On-chip layout is feature-on-partition ("transposed activations"):
activations live as [feat, batch] so every layer is a natural
weight-stationary matmul  hT_L = W_L.T @ hT_{L-1}  with contraction on the
partition dim.  x is transposed host-side so no on-chip transpose is needed.

The per-sample gram matrices are computed by packing 8 samples into one
128x128 matmul (columns = (sample, field)); the 8 useful 16x16 diagonal
blocks are pulled out through a DRAM scratch roundtrip with a strided
gather, directly into [pair, batch] layout.  The upper-triangle selection
is folded into the classifier weights host-side (rows for i>=j get zeros),
so the final layer contracts over all 256 (i,j) cells plus the 64 deep
features in one accumulated matmul chain.
"""

import os
import numpy as np
from contextlib import ExitStack

import concourse.bass as bass
import concourse.bacc as bacc
import concourse.mybir as mybir
import concourse.tile as tile
from concourse.bass_utils import run_bass_kernel_spmd
from concourse.masks import make_identity

# Problem shape (hardcoded per contest rules).
B, N, D = 4096, 16, 256
ND = N * D              # 4096
H1, H2, HO = 1024, 512, 64
PAIRS = N * (N - 1) // 2
C = 5
NCORES = 8
BL = B // NCORES        # 512 rows per core

P = 128
F32 = mybir.dt.float32
F32R = mybir.dt.float32r
BF16 = mybir.dt.bfloat16

AFT = mybir.ActivationFunctionType

# dtype knob for the dense MLP matmuls (layers 1-3): bf16 halves weight
# DMA traffic and runs the PE at 1 cycle/row (fp32 runs at 4).
MLP_BF16 = True
MLP_DT = BF16 if MLP_BF16 else F32

K1 = ND // P        # 32 k-tiles for layer 1
OT1 = H1 // P       # 8 output tiles layer 1
K2 = H1 // P        # 8
OT2 = H2 // P       # 4
K3 = H2 // P        # 4
NGROUP = BL // 8    # 64 gram groups of 8 samples
GCHUNK = 8          # gram groups per extraction chunk
NCHUNK = NGROUP // GCHUNK  # 8


def _mm_view(ap, dt):
    return ap.bitcast(dt) if dt != ap.dtype else ap


def _build_program():
    nc = bacc.Bacc(
        "TRN2",
        target_bir_lowering=False,
        debug=False,
        num_devices=NCORES,
    )

    # x shard in two bf16 layouts:
    #   xg[t, dlocal, b, i] = x[b, i, t*128+dlocal]  (gram: contiguous
    #     [128,128] operands per 8-sample group)
    #   xl1[kt, dlocal, b] = k-tile-major x^T  (layer-1: contiguous rhs)
    # all bulk operands are partition-major: [.., p, <per-partition run>]
    # so every DMA is 2-dim with multi-KB contiguous packets
    xg_d = nc.dram_tensor("xg", [P, 2, BL, N], BF16, kind="ExternalInput")
    xl1_d = nc.dram_tensor("xl1", [P, K1, BL], BF16, kind="ExternalInput")
    w1_d = nc.dram_tensor("w1r", [OT1, P, K1, P], MLP_DT, kind="ExternalInput")
    w2_d = nc.dram_tensor("w2r", [OT2, P, K2, P], MLP_DT, kind="ExternalInput")
    w3_d = nc.dram_tensor("w3", [P, K3, HO], MLP_DT, kind="ExternalInput")
    b1_d = nc.dram_tensor("b1", [H1], F32, kind="ExternalInput")
    b2_d = nc.dram_tensor("b2", [H2], F32, kind="ExternalInput")
    b3_d = nc.dram_tensor("b3", [HO], F32, kind="ExternalInput")
    # classifier, host-padded: wca [128,5] (64 deep rows + zeros),
    # wcb/wcc [128,5] for gram cells i in 0..7 / 8..15 (zeros where i>=j)
    wca_d = nc.dram_tensor("wca", [P, C], F32, kind="ExternalInput")
    wcb_d = nc.dram_tensor("wcb", [P, C], F32, kind="ExternalInput")
    wcc_d = nc.dram_tensor("wcc", [P, C], F32, kind="ExternalInput")
    bc_d = nc.dram_tensor("bc", [C], F32, kind="ExternalInput")
    out_d = nc.dram_tensor("out", [C, BL], F32, kind="ExternalOutput")

    with tile.TileContext(nc) as tc:
        with ExitStack() as ctx:
            _kernel_body(
                ctx, tc,
                xg_d, xl1_d, w1_d, w2_d, w3_d, b1_d, b2_d, b3_d,
                wca_d, wcb_d, wcc_d, bc_d, out_d,
            )
    nc.compile()
    return nc


def _kernel_body(ctx, tc, xg_d, xl1_d, w1_d, w2_d, w3_d, b1_d, b2_d, b3_d,
                 wca_d, wcb_d, wcc_d, bc_d, out_d):
    nc = tc.nc

    pool_const = ctx.enter_context(tc.tile_pool(name="const", bufs=1))
    pool_xt = ctx.enter_context(tc.tile_pool(name="xt", bufs=1))
    pool_w1 = ctx.enter_context(tc.tile_pool(name="w1", bufs=2))
    pool_h1 = ctx.enter_context(tc.tile_pool(name="h1", bufs=1))
    pool_h2 = ctx.enter_context(tc.tile_pool(name="h2", bufs=1))
    pool_gram = ctx.enter_context(tc.tile_pool(name="gram", bufs=2))
    pool_gramT = ctx.enter_context(tc.tile_pool(name="gramT", bufs=1))
    pool_psum = ctx.enter_context(tc.tile_pool(name="psum", bufs=2, space="PSUM"))
    pool_psum_g = ctx.enter_context(
        tc.tile_pool(name="psum_g", bufs=2, space="PSUM"))
    pool_dram = ctx.enter_context(tc.tile_pool(name="scratch", bufs=1, space="DRAM"))

    # ---- constants / weights ----
    b1_sb = pool_const.tile([P, OT1], F32)
    nc.scalar.dma_start(b1_sb[:], b1_d.ap().rearrange("(o p) -> p o", p=P))
    b2_sb = pool_const.tile([P, OT2], F32)
    nc.scalar.dma_start(b2_sb[:], b2_d.ap().rearrange("(o p) -> p o", p=P))
    b3_sb = pool_const.tile([HO, 1], F32)
    nc.scalar.dma_start(b3_sb[:], b3_d.ap().rearrange("(p o) -> p o", o=1))
    bc_sb = pool_const.tile([C, 1], F32)
    nc.scalar.dma_start(bc_sb[:], bc_d.ap().rearrange("(p o) -> p o", o=1))
    wca_sb = pool_const.tile([P, C], F32)
    nc.scalar.dma_start(wca_sb[:], wca_d.ap())
    wcb_sb = pool_const.tile([P, C], F32)
    nc.scalar.dma_start(wcb_sb[:], wcb_d.ap())
    wcc_sb = pool_const.tile([P, C], F32)
    nc.scalar.dma_start(wcc_sb[:], wcc_d.ap())

    # W2 slabs: [H1, 128] per output tile, resident
    w2_sb = pool_const.tile([P, OT2, K2, P], MLP_DT)
    for o2 in range(OT2):
        nc.scalar.dma_start(w2_sb[:, o2], w2_d.ap()[o2])
    # W3: [H2, 64] resident
    w3_sb = pool_const.tile([P, K3, HO], MLP_DT)
    nc.scalar.dma_start(w3_sb[:], w3_d.ap())

    # ---- x (both layouts) ----
    xg_sb = pool_xt.tile([P, 2, BL, N], BF16)
    for t in range(2):
        for bc4 in range(2):
            nc.sync.dma_start(
                xg_sb[:, t, bc4 * 256:(bc4 + 1) * 256],
                xg_d.ap()[:, t, bc4 * 256:(bc4 + 1) * 256],
            )
    xl1_sb = pool_xt.tile([P, K1, BL], BF16)
    for k4 in range(8):
        nc.sync.dma_start(
            xl1_sb[:, k4 * 4:(k4 + 1) * 4],
            xl1_d.ap()[:, k4 * 4:(k4 + 1) * 4],
        )

    def l1_rhs(kt):
        return xl1_sb[:, kt]

    # ---- layer-1 slab loader ----
    def load_w1_slab(ot):
        slab = pool_w1.tile([P, K1, P], MLP_DT, tag="w1slab")
        for cch in range(4):
            k0 = cch * 8
            nc.sync.dma_start(
                slab[:, k0:k0 + 8], w1_d.ap()[ot, :, k0:k0 + 8])
        return slab

    h1_sb = pool_h1.tile([P, K2, BL], MLP_DT)

    def l1_pass(ot, slab):
        ps = pool_psum.tile([P, BL], F32, tag="ps_mlp")
        for kt in range(K1):
            nc.tensor.matmul(
                ps[:],
                slab[:, kt],
                l1_rhs(kt),
                start=(kt == 0),
                stop=(kt == K1 - 1),
            )
        nc.scalar.activation(
            h1_sb[:, ot], ps[:], AFT.Relu, bias=b1_sb[:, ot:ot + 1])

    # ---- layer 1, first pass (overlaps with x^T arrival) ----
    slab0 = load_w1_slab(0)
    l1_pass(0, slab0)

    # ---- gram: 8 samples per 128x128 bf16 matmul ----
    def gram_operand(g, t):
        # columns m = s*16 + i are contiguous in the xg layout
        return xg_sb[:, t, g * 8:(g + 1) * 8, :]

    # scratch[p, g*128 + f] — partition-major so the scatter writes 4KB runs
    scratch = pool_dram.tile([P * NGROUP * P], F32)

    gram_chunks = []
    for ch in range(NCHUNK):
        gbuf = pool_gram.tile([P, GCHUNK, P], F32, tag="gbuf")
        for gl in range(GCHUNK):
            g = ch * GCHUNK + gl
            pg = pool_psum_g.tile([P, P], F32, tag="ps_gram")
            for t in range(2):
                nc.tensor.matmul(
                    pg[:],
                    gram_operand(g, t),
                    gram_operand(g, t),
                    start=(t == 0),
                    stop=(t == 1),
                )
            nc.vector.tensor_copy(gbuf[:, gl], pg[:])
        # scatter chunk to DRAM scratch: addr = p*(NGROUP*128) + g*128 + f
        nc.scalar.dma_start(
            bass.AP(scratch.tensor,
                    scratch.offset + ch * GCHUNK * P,
                    [[NGROUP * P, P], [1, GCHUNK * P]]),
            gbuf[:],
        )
        gram_chunks.append(gbuf)

    # gather diagonal 16x16 blocks into gramT halves [128, BL]
    # dst partition = (i%8)*16 + j, free = g*8 + s
    # src elem addr = g*16384 + s*2064 + i*128 + j  (within chunk)
    # gather diagonal 16x16 blocks into natural layout G_sb[b, (i,j)]:
    # one DMA per group, dims (s, i, j), b-contiguous on both sides
    g_sb = pool_gramT.tile([P, 4, N * N], F32, tag="g_nat", name="g_nat")
    RW = NGROUP * P  # scratch row width (per-partition elements)
    for g in range(NGROUP):
        # elem addr = (s*16+i)*RW + g*128 + s*16 + j
        src = bass.AP(
            scratch.tensor,
            scratch.offset + g * P,
            [[16 * RW + 16, 8], [RW, N], [1, N]],
        )
        dst = bass.AP(
            g_sb.tensor,
            g_sb.offset + (g % 16) * 8 * (4 * N * N) + (g // 16) * N * N,
            [[4 * N * N, 8], [N, N], [1, N]],
        )
        nc.sync.dma_start(dst, src)

    # PE-transpose 128x128 blocks of G_sb into gramT[h] = [cells, b]
    identity = pool_const.tile([P, P], F32)
    make_identity(nc, identity)
    gramT = [
        pool_gramT.tile([P, BL], F32, tag=f"gramT{h}", name=f"gramT{h}")
        for h in range(2)
    ]
    for bt in range(4):
        for h in range(2):
            pt = pool_psum_g.tile([P, P], F32, tag="ps_tr", name="ps_tr")
            nc.tensor.transpose(
                pt[:], g_sb[:, bt, h * P:(h + 1) * P], identity[:])
            nc.vector.tensor_copy(gramT[h][:, bt * P:(bt + 1) * P], pt[:])

    # ---- layer 1, remaining passes ----
    for ot in range(1, OT1):
        slab = load_w1_slab(ot)
        l1_pass(ot, slab)

    # ---- layer 2 ----
    h2_sb = pool_h2.tile([P, K3, BL], MLP_DT)
    for o2 in range(OT2):
        ps = pool_psum.tile([P, BL], F32, tag="ps_mlp")
        for kt in range(K2):
            nc.tensor.matmul(
                ps[:],
                w2_sb[:, o2, kt],
                h1_sb[:, kt],
                start=(kt == 0),
                stop=(kt == K2 - 1),
            )
        nc.scalar.activation(
            h2_sb[:, o2], ps[:], AFT.Relu, bias=b2_sb[:, o2:o2 + 1])

    # ---- layer 3 -> higher [64, BL] padded to 128 rows of zeros ----
    higher_sb = pool_gramT.tile([P, BL], F32, tag="higher")
    nc.gpsimd.memset(higher_sb[HO:, :], 0.0)
    ps3 = pool_psum.tile([P, BL], F32, tag="ps_mlp")
    for kt in range(K3):
        nc.tensor.matmul(
            ps3[:HO],
            w3_sb[:, kt],
            h2_sb[:, kt],
            start=(kt == 0),
            stop=(kt == K3 - 1),
        )
    nc.scalar.activation(
        higher_sb[:HO], ps3[:HO], AFT.Identity, bias=b3_sb[:])

    # ---- classifier: out[5, BL] = Wc.T @ [higher; gram cells] + bc ----
    psf = pool_psum.tile([P, BL], F32, tag="ps_mlp")
    nc.tensor.matmul(psf[:C], wca_sb[:], higher_sb[:], start=True, stop=False)
    nc.tensor.matmul(psf[:C], wcb_sb[:], gramT[0][:], start=False, stop=False)
    nc.tensor.matmul(psf[:C], wcc_sb[:], gramT[1][:], start=False, stop=True)
    out_sb = pool_const.tile([C, BL], F32)
    nc.scalar.activation(out_sb[:], psf[:C], AFT.Identity, bias=bc_sb[:])
    nc.sync.dma_start(out_d.ap(), out_sb[:])


_CACHED = None


def _get_program():
    global _CACHED
    if _CACHED is None:
        _CACHED = _build_program()
    return _CACHED


def prepare_in_maps(x, W1, b1, W2, b2, W3, b3, Wc, bc):
    x = np.ascontiguousarray(np.asarray(x, dtype=np.float32))
    W1 = np.asarray(W1, dtype=np.float32)
    W2 = np.asarray(W2, dtype=np.float32)
    W3 = np.ascontiguousarray(np.asarray(W3, dtype=np.float32))
    b1 = np.asarray(b1, dtype=np.float32)
    b2 = np.asarray(b2, dtype=np.float32)
    b3 = np.asarray(b3, dtype=np.float32)
    Wc = np.asarray(Wc, dtype=np.float32)
    bc = np.asarray(bc, dtype=np.float32)

    # host-side layout prep (replicated operands)
    import ml_dtypes
    wdt = ml_dtypes.bfloat16 if MLP_BF16 else np.float32
    # [ot, p, kt, o] / [ot2, p, kt, o] / [p, kt, o] partition-major
    w1r = np.ascontiguousarray(
        W1.reshape(K1, P, OT1, P).transpose(2, 1, 0, 3).astype(wdt))
    w2r = np.ascontiguousarray(
        W2.reshape(K2, P, OT2, P).transpose(2, 1, 0, 3).astype(wdt))
    W3 = np.ascontiguousarray(
        W3.reshape(K3, P, HO).transpose(1, 0, 2).astype(wdt))

    # classifier padding: fold triu-pair selection into gram-cell weights
    iu, ju = np.triu_indices(N, k=1)
    wc_gram = np.zeros((N * N, C), dtype=np.float32)
    wc_gram[iu * N + ju] = Wc[HO:]
    wca = np.zeros((P, C), dtype=np.float32)
    wca[:HO] = Wc[:HO]
    wcb = np.ascontiguousarray(wc_gram[:P])
    wcc = np.ascontiguousarray(wc_gram[P:])

    common = dict(
        w1r=w1r, w2r=w2r, w3=W3, b1=b1, b2=b2, b3=b3,
        wca=wca, wcb=wcb, wcc=wcc, bc=bc,
    )
    bf = ml_dtypes.bfloat16
    maps = []
    for c in range(NCORES):
        xs = x[c * BL:(c + 1) * BL].reshape(BL, N, 2, P).astype(bf)
        # xg[p, t, b, i];  xl1[p, kt, b] with kt = 2*i + t
        maps.append(dict(
            common,
            xg=np.ascontiguousarray(xs.transpose(3, 2, 0, 1)),
            xl1=np.ascontiguousarray(
                xs.transpose(3, 1, 2, 0).reshape(P, K1, BL)),
        ))
    return maps


def kernel(x, W1, b1, W2, b2, W3, b3, Wc, bc):
    in_maps = prepare_in_maps(x, W1, b1, W2, b2, W3, b3, Wc, bc)
    nc = _get_program()
    res = run_bass_kernel_spmd(nc, in_maps, core_ids=list(range(NCORES)))
    out = np.empty((B, C), dtype=np.float32)
    for c in range(NCORES):
        out[c * BL:(c + 1) * BL] = res.results[c]["out"].T
    return out


# revision 19
# speedup vs baseline: 4.1239x; 1.2107x over previous
"""DeepFM forward kernel for Trainium2, data-parallel over 8 NeuronCores.

Model (B=4096, N=16, D=256):
  gram[b]   = x[b] @ x[b].T                       (second-order interactions)
  h1        = relu(flat(x) @ W1 + b1)             (4096 -> 1024)
  h2        = relu(h1 @ W2 + b2)                  (1024 -> 512)
  higher    = h2 @ W3 + b3                        (512 -> 64)
  out       = [higher, triu(gram)] @ Wc + bc      (184 -> 5)

Sharding: batch split 8 ways (512 rows/core), all weights replicated.

On-chip layout is feature-on-partition ("transposed activations"):
activations live as [feat, batch] so every layer is a natural
weight-stationary matmul  hT_L = W_L.T @ hT_{L-1}  with contraction on the
partition dim.  x is transposed host-side so no on-chip transpose is needed.

The per-sample gram matrices are computed by packing 8 samples into one
128x128 matmul (columns = (sample, field)); the 8 useful 16x16 diagonal
blocks are pulled out through a DRAM scratch roundtrip with a strided
gather, directly into [pair, batch] layout.  The upper-triangle selection
is folded into the classifier weights host-side (rows for i>=j get zeros),
so the final layer contracts over all 256 (i,j) cells plus the 64 deep
features in one accumulated matmul chain.
"""

import os
import numpy as np
from contextlib import ExitStack

import concourse.bass as bass
import concourse.bacc as bacc
import concourse.mybir as mybir
import concourse.tile as tile
from concourse.bass_utils import run_bass_kernel_spmd
from concourse.masks import make_identity

# Problem shape (hardcoded per contest rules).
B, N, D = 4096, 16, 256
ND = N * D              # 4096
H1, H2, HO = 1024, 512, 64
PAIRS = N * (N - 1) // 2
C = 5
NCORES = 8
BL = B // NCORES        # 512 rows per core

P = 128
F32 = mybir.dt.float32
F32R = mybir.dt.float32r
BF16 = mybir.dt.bfloat16

AFT = mybir.ActivationFunctionType

# dtype knob for the dense MLP matmuls (layers 1-3): bf16 halves weight
# DMA traffic and runs the PE at 1 cycle/row (fp32 runs at 4).
MLP_BF16 = True
MLP_DT = BF16 if MLP_BF16 else F32

K1 = ND // P        # 32 k-tiles for layer 1
OT1 = H1 // P       # 8 output tiles layer 1
K2 = H1 // P        # 8
OT2 = H2 // P       # 4
K3 = H2 // P        # 4
NGROUP = BL // 8    # 64 gram groups of 8 samples
GCHUNK = 8          # gram groups per extraction chunk
NCHUNK = NGROUP // GCHUNK  # 8


def _mm_view(ap, dt):
    return ap.bitcast(dt) if dt != ap.dtype else ap


def _build_program():
    nc = bacc.Bacc(
        "TRN2",
        target_bir_lowering=False,
        debug=False,
        num_devices=NCORES,
    )

    # x shard in two bf16 layouts:
    #   xg[t, dlocal, b, i] = x[b, i, t*128+dlocal]  (gram: contiguous
    #     [128,128] operands per 8-sample group)
    #   xl1[kt, dlocal, b] = k-tile-major x^T  (layer-1: contiguous rhs)
    # all bulk operands are partition-major: [.., p, <per-partition run>]
    # so every DMA is 2-dim with multi-KB contiguous packets
    xg_d = nc.dram_tensor("xg", [P, 2, BL, N], BF16, kind="ExternalInput")
    xl1_d = nc.dram_tensor("xl1", [P, K1, BL], BF16, kind="ExternalInput")
    w1_d = nc.dram_tensor("w1r", [OT1, P, K1, P], MLP_DT, kind="ExternalInput")
    w2_d = nc.dram_tensor("w2r", [OT2, P, K2, P], MLP_DT, kind="ExternalInput")
    w3_d = nc.dram_tensor("w3", [P, K3, HO], MLP_DT, kind="ExternalInput")
    b1_d = nc.dram_tensor("b1", [H1], F32, kind="ExternalInput")
    b2_d = nc.dram_tensor("b2", [H2], F32, kind="ExternalInput")
    b3_d = nc.dram_tensor("b3", [HO], F32, kind="ExternalInput")
    # classifier, host-padded: wca [128,5] (64 deep rows + zeros),
    # wcb/wcc [128,5] for gram cells i in 0..7 / 8..15 (zeros where i>=j)
    wca_d = nc.dram_tensor("wca", [P, C], F32, kind="ExternalInput")
    wcb_d = nc.dram_tensor("wcb", [P, C], F32, kind="ExternalInput")
    wcc_d = nc.dram_tensor("wcc", [P, C], F32, kind="ExternalInput")
    bc_d = nc.dram_tensor("bc", [C], F32, kind="ExternalInput")
    out_d = nc.dram_tensor("out", [C, BL], F32, kind="ExternalOutput")

    with tile.TileContext(nc) as tc:
        with ExitStack() as ctx:
            _kernel_body(
                ctx, tc,
                xg_d, xl1_d, w1_d, w2_d, w3_d, b1_d, b2_d, b3_d,
                wca_d, wcb_d, wcc_d, bc_d, out_d,
            )
    nc.compile()
    return nc


def _kernel_body(ctx, tc, xg_d, xl1_d, w1_d, w2_d, w3_d, b1_d, b2_d, b3_d,
                 wca_d, wcb_d, wcc_d, bc_d, out_d):
    nc = tc.nc

    pool_const = ctx.enter_context(tc.tile_pool(name="const", bufs=1))
    pool_xt = ctx.enter_context(tc.tile_pool(name="xt", bufs=1))
    pool_w1 = ctx.enter_context(tc.tile_pool(name="w1", bufs=2))
    pool_h1 = ctx.enter_context(tc.tile_pool(name="h1", bufs=1))
    pool_h2 = ctx.enter_context(tc.tile_pool(name="h2", bufs=1))
    pool_gram = ctx.enter_context(tc.tile_pool(name="gram", bufs=2))
    pool_gramT = ctx.enter_context(tc.tile_pool(name="gramT", bufs=1))
    pool_psum = ctx.enter_context(tc.tile_pool(name="psum", bufs=2, space="PSUM"))
    pool_psum_g = ctx.enter_context(
        tc.tile_pool(name="psum_g", bufs=2, space="PSUM"))
    pool_dram = ctx.enter_context(tc.tile_pool(name="scratch", bufs=1, space="DRAM"))

    # ---- constants / weights ----
    b1_sb = pool_const.tile([P, OT1], F32)
    nc.scalar.dma_start(b1_sb[:], b1_d.ap().rearrange("(o p) -> p o", p=P))
    b2_sb = pool_const.tile([P, OT2], F32)
    nc.scalar.dma_start(b2_sb[:], b2_d.ap().rearrange("(o p) -> p o", p=P))
    b3_sb = pool_const.tile([HO, 1], F32)
    nc.scalar.dma_start(b3_sb[:], b3_d.ap().rearrange("(p o) -> p o", o=1))
    bc_sb = pool_const.tile([C, 1], F32)
    nc.scalar.dma_start(bc_sb[:], bc_d.ap().rearrange("(p o) -> p o", o=1))
    wca_sb = pool_const.tile([P, C], F32)
    nc.scalar.dma_start(wca_sb[:], wca_d.ap())
    wcb_sb = pool_const.tile([P, C], F32)
    nc.scalar.dma_start(wcb_sb[:], wcb_d.ap())
    wcc_sb = pool_const.tile([P, C], F32)
    nc.scalar.dma_start(wcc_sb[:], wcc_d.ap())

    # W2 slabs: [H1, 128] per output tile, resident
    w2_sb = pool_const.tile([P, OT2, K2, P], MLP_DT)
    for o2 in range(OT2):
        nc.scalar.dma_start(w2_sb[:, o2], w2_d.ap()[o2])
    # W3: [H2, 64] resident
    w3_sb = pool_const.tile([P, K3, HO], MLP_DT)
    nc.scalar.dma_start(w3_sb[:], w3_d.ap())

    # ---- x (both layouts) ----
    xg_sb = pool_xt.tile([P, 2, BL, N], BF16)
    for bc4 in range(2):
        for t in range(2):
            nc.sync.dma_start(
                xg_sb[:, t, bc4 * 256:(bc4 + 1) * 256],
                xg_d.ap()[:, t, bc4 * 256:(bc4 + 1) * 256],
            )
    xl1_sb = pool_xt.tile([P, K1, BL], BF16)
    for k4 in range(8):
        nc.sync.dma_start(
            xl1_sb[:, k4 * 4:(k4 + 1) * 4],
            xl1_d.ap()[:, k4 * 4:(k4 + 1) * 4],
        )

    def l1_rhs(kt):
        return xl1_sb[:, kt]

    # ---- layer-1 slab loader ----
    def load_w1_slab(ot):
        slab = pool_w1.tile([P, K1, P], MLP_DT, tag="w1slab")
        for cch in range(4):
            k0 = cch * 8
            nc.sync.dma_start(
                slab[:, k0:k0 + 8], w1_d.ap()[ot, :, k0:k0 + 8])
        return slab

    h1_sb = pool_h1.tile([P, K2, BL], MLP_DT)

    def l1_pass(ot, slab):
        ps = pool_psum.tile([P, BL], F32, tag="ps_mlp")
        for kt in range(K1):
            nc.tensor.matmul(
                ps[:],
                slab[:, kt],
                l1_rhs(kt),
                start=(kt == 0),
                stop=(kt == K1 - 1),
            )
        nc.scalar.activation(
            h1_sb[:, ot], ps[:], AFT.Relu, bias=b1_sb[:, ot:ot + 1])

    # ---- gram: 8 samples per 128x128 bf16 matmul ----
    def gram_operand(g, t):
        # columns m = s*16 + i are contiguous in the xg layout
        return xg_sb[:, t, g * 8:(g + 1) * 8, :]

    # scratch[p, g*128 + f] — partition-major so the scatter writes 4KB runs
    scratch = pool_dram.tile([P * NGROUP * P], F32)

    gram_chunks = []
    for ch in range(NCHUNK):
        gbuf = pool_gram.tile([P, GCHUNK, P], F32, tag="gbuf")
        for gl in range(GCHUNK):
            g = ch * GCHUNK + gl
            pg = pool_psum_g.tile([P, P], F32, tag="ps_gram")
            for t in range(2):
                nc.tensor.matmul(
                    pg[:],
                    gram_operand(g, t),
                    gram_operand(g, t),
                    start=(t == 0),
                    stop=(t == 1),
                )
            nc.vector.tensor_copy(gbuf[:, gl], pg[:])
        # scatter chunk to DRAM scratch: addr = p*(NGROUP*128) + g*128 + f
        nc.scalar.dma_start(
            bass.AP(scratch.tensor,
                    scratch.offset + ch * GCHUNK * P,
                    [[NGROUP * P, P], [1, GCHUNK * P]]),
            gbuf[:],
        )
        gram_chunks.append(gbuf)

    # gather diagonal 16x16 blocks into gramT halves [128, BL]
    # dst partition = (i%8)*16 + j, free = g*8 + s
    # src elem addr = g*16384 + s*2064 + i*128 + j  (within chunk)
    # gather diagonal 16x16 blocks into natural layout G_sb[b, (i,j)]:
    # one DMA per group, dims (s, i, j), b-contiguous on both sides
    g_sb = pool_gramT.tile([P, 4, N * N], F32, tag="g_nat", name="g_nat")
    RW = NGROUP * P  # scratch row width (per-partition elements)
    for g in range(NGROUP):
        # elem addr = (s*16+i)*RW + g*128 + s*16 + j
        src = bass.AP(
            scratch.tensor,
            scratch.offset + g * P,
            [[16 * RW + 16, 8], [RW, N], [1, N]],
        )
        dst = bass.AP(
            g_sb.tensor,
            g_sb.offset + (g % 16) * 8 * (4 * N * N) + (g // 16) * N * N,
            [[4 * N * N, 8], [N, N], [1, N]],
        )
        nc.gpsimd.dma_start(dst, src)

    # ---- layer 1 ----
    for ot in range(OT1):
        slab = load_w1_slab(ot)
        l1_pass(ot, slab)

    # PE-transpose 128x128 blocks of G_sb into gramT[h] = [cells, b]
    identity = pool_const.tile([P, P], F32)
    make_identity(nc, identity)
    gramT = [
        pool_gramT.tile([P, BL], F32, tag=f"gramT{h}", name=f"gramT{h}")
        for h in range(2)
    ]
    for bt in range(4):
        for h in range(2):
            pt = pool_psum_g.tile([P, P], F32, tag="ps_tr", name="ps_tr")
            nc.tensor.transpose(
                pt[:], g_sb[:, bt, h * P:(h + 1) * P], identity[:])
            nc.vector.tensor_copy(gramT[h][:, bt * P:(bt + 1) * P], pt[:])

    # ---- layer 2 ----
    h2_sb = pool_h2.tile([P, K3, BL], MLP_DT)
    for o2 in range(OT2):
        ps = pool_psum.tile([P, BL], F32, tag="ps_mlp")
        for kt in range(K2):
            nc.tensor.matmul(
                ps[:],
                w2_sb[:, o2, kt],
                h1_sb[:, kt],
                start=(kt == 0),
                stop=(kt == K2 - 1),
            )
        nc.scalar.activation(
            h2_sb[:, o2], ps[:], AFT.Relu, bias=b2_sb[:, o2:o2 + 1])

    # ---- layer 3 -> higher [64, BL] padded to 128 rows of zeros ----
    higher_sb = pool_gramT.tile([P, BL], F32, tag="higher")
    nc.gpsimd.memset(higher_sb[HO:, :], 0.0)
    ps3 = pool_psum.tile([P, BL], F32, tag="ps_mlp")
    for kt in range(K3):
        nc.tensor.matmul(
            ps3[:HO],
            w3_sb[:, kt],
            h2_sb[:, kt],
            start=(kt == 0),
            stop=(kt == K3 - 1),
        )
    nc.scalar.activation(
        higher_sb[:HO], ps3[:HO], AFT.Identity, bias=b3_sb[:])

    # ---- classifier: out[5, BL] = Wc.T @ [higher; gram cells] + bc ----
    psf = pool_psum.tile([P, BL], F32, tag="ps_mlp")
    nc.tensor.matmul(psf[:C], wca_sb[:], higher_sb[:], start=True, stop=False)
    nc.tensor.matmul(psf[:C], wcb_sb[:], gramT[0][:], start=False, stop=False)
    nc.tensor.matmul(psf[:C], wcc_sb[:], gramT[1][:], start=False, stop=True)
    out_sb = pool_const.tile([C, BL], F32)
    nc.scalar.activation(out_sb[:], psf[:C], AFT.Identity, bias=bc_sb[:])
    nc.sync.dma_start(out_d.ap(), out_sb[:])


_CACHED = None


def _get_program():
    global _CACHED
    if _CACHED is None:
        _CACHED = _build_program()
    return _CACHED


def prepare_in_maps(x, W1, b1, W2, b2, W3, b3, Wc, bc):
    x = np.ascontiguousarray(np.asarray(x, dtype=np.float32))
    W1 = np.asarray(W1, dtype=np.float32)
    W2 = np.asarray(W2, dtype=np.float32)
    W3 = np.ascontiguousarray(np.asarray(W3, dtype=np.float32))
    b1 = np.asarray(b1, dtype=np.float32)
    b2 = np.asarray(b2, dtype=np.float32)
    b3 = np.asarray(b3, dtype=np.float32)
    Wc = np.asarray(Wc, dtype=np.float32)
    bc = np.asarray(bc, dtype=np.float32)

    # host-side layout prep (replicated operands)
    import ml_dtypes
    wdt = ml_dtypes.bfloat16 if MLP_BF16 else np.float32
    # [ot, p, kt, o] / [ot2, p, kt, o] / [p, kt, o] partition-major
    w1r = np.ascontiguousarray(
        W1.reshape(K1, P, OT1, P).transpose(2, 1, 0, 3).astype(wdt))
    w2r = np.ascontiguousarray(
        W2.reshape(K2, P, OT2, P).transpose(2, 1, 0, 3).astype(wdt))
    W3 = np.ascontiguousarray(
        W3.reshape(K3, P, HO).transpose(1, 0, 2).astype(wdt))

    # classifier padding: fold triu-pair selection into gram-cell weights
    iu, ju = np.triu_indices(N, k=1)
    wc_gram = np.zeros((N * N, C), dtype=np.float32)
    wc_gram[iu * N + ju] = Wc[HO:]
    wca = np.zeros((P, C), dtype=np.float32)
    wca[:HO] = Wc[:HO]
    wcb = np.ascontiguousarray(wc_gram[:P])
    wcc = np.ascontiguousarray(wc_gram[P:])

    common = dict(
        w1r=w1r, w2r=w2r, w3=W3, b1=b1, b2=b2, b3=b3,
        wca=wca, wcb=wcb, wcc=wcc, bc=bc,
    )
    bf = ml_dtypes.bfloat16
    maps = []
    for c in range(NCORES):
        xs = x[c * BL:(c + 1) * BL].reshape(BL, N, 2, P).astype(bf)
        # xg[p, t, b, i];  xl1[p, kt, b] with kt = 2*i + t
        maps.append(dict(
            common,
            xg=np.ascontiguousarray(xs.transpose(3, 2, 0, 1)),
            xl1=np.ascontiguousarray(
                xs.transpose(3, 1, 2, 0).reshape(P, K1, BL)),
        ))
    return maps


def kernel(x, W1, b1, W2, b2, W3, b3, Wc, bc):
    in_maps = prepare_in_maps(x, W1, b1, W2, b2, W3, b3, Wc, bc)
    nc = _get_program()
    res = run_bass_kernel_spmd(nc, in_maps, core_ids=list(range(NCORES)))
    out = np.empty((B, C), dtype=np.float32)
    for c in range(NCORES):
        out[c * BL:(c + 1) * BL] = res.results[c]["out"].T
    return out


# revision 20
# speedup vs baseline: 4.3835x; 1.0630x over previous
"""DeepFM forward kernel for Trainium2, data-parallel over 8 NeuronCores.

Model (B=4096, N=16, D=256):
  gram[b]   = x[b] @ x[b].T                       (second-order interactions)
  h1        = relu(flat(x) @ W1 + b1)             (4096 -> 1024)
  h2        = relu(h1 @ W2 + b2)                  (1024 -> 512)
  higher    = h2 @ W3 + b3                        (512 -> 64)
  out       = [higher, triu(gram)] @ Wc + bc      (184 -> 5)

Sharding: batch split 8 ways (512 rows/core), all weights replicated.

On-chip layout is feature-on-partition ("transposed activations"):
activations live as [feat, batch] so every layer is a natural
weight-stationary matmul  hT_L = W_L.T @ hT_{L-1}  with contraction on the
partition dim.  x is transposed host-side so no on-chip transpose is needed.

The per-sample gram matrices are computed by packing 8 samples into one
128x128 matmul (columns = (sample, field)); the 8 useful 16x16 diagonal
blocks are pulled out through a DRAM scratch roundtrip with a strided
gather, directly into [pair, batch] layout.  The upper-triangle selection
is folded into the classifier weights host-side (rows for i>=j get zeros),
so the final layer contracts over all 256 (i,j) cells plus the 64 deep
features in one accumulated matmul chain.
"""

import os
import numpy as np
from contextlib import ExitStack

import concourse.bass as bass
import concourse.bacc as bacc
import concourse.mybir as mybir
import concourse.tile as tile
from concourse.bass_utils import run_bass_kernel_spmd
from concourse.masks import make_identity

# Problem shape (hardcoded per contest rules).
B, N, D = 4096, 16, 256
ND = N * D              # 4096
H1, H2, HO = 1024, 512, 64
PAIRS = N * (N - 1) // 2
C = 5
NCORES = 8
BL = B // NCORES        # 512 rows per core

P = 128
F32 = mybir.dt.float32
F32R = mybir.dt.float32r
BF16 = mybir.dt.bfloat16

AFT = mybir.ActivationFunctionType

# dtype knob for the dense MLP matmuls (layers 1-3): bf16 halves weight
# DMA traffic and runs the PE at 1 cycle/row (fp32 runs at 4).
MLP_BF16 = True
MLP_DT = BF16 if MLP_BF16 else F32

K1 = ND // P        # 32 k-tiles for layer 1
OT1 = H1 // P       # 8 output tiles layer 1
K2 = H1 // P        # 8
OT2 = H2 // P       # 4
K3 = H2 // P        # 4
NGROUP = BL // 8    # 64 gram groups of 8 samples
GCHUNK = 8          # gram groups per extraction chunk
NCHUNK = NGROUP // GCHUNK  # 8


def _mm_view(ap, dt):
    return ap.bitcast(dt) if dt != ap.dtype else ap


def _build_program():
    nc = bacc.Bacc(
        "TRN2",
        target_bir_lowering=False,
        debug=False,
        num_devices=NCORES,
    )

    # x shard in two bf16 layouts:
    #   xg[t, dlocal, b, i] = x[b, i, t*128+dlocal]  (gram: contiguous
    #     [128,128] operands per 8-sample group)
    #   xl1[kt, dlocal, b] = k-tile-major x^T  (layer-1: contiguous rhs)
    # all bulk operands are partition-major: [.., p, <per-partition run>]
    # so every DMA is 2-dim with multi-KB contiguous packets
    xg_d = nc.dram_tensor("xg", [P, 2, BL, N], BF16, kind="ExternalInput")
    xl1_d = nc.dram_tensor("xl1", [P, K1, BL], BF16, kind="ExternalInput")
    w1_d = nc.dram_tensor("w1r", [OT1, P, K1, P], MLP_DT, kind="ExternalInput")
    w2_d = nc.dram_tensor("w2r", [OT2, P, K2, P], MLP_DT, kind="ExternalInput")
    w3_d = nc.dram_tensor("w3", [P, K3, HO], MLP_DT, kind="ExternalInput")
    b1_d = nc.dram_tensor("b1", [H1], F32, kind="ExternalInput")
    b2_d = nc.dram_tensor("b2", [H2], F32, kind="ExternalInput")
    b3_d = nc.dram_tensor("b3", [HO], F32, kind="ExternalInput")
    # classifier, host-padded: wca [128,5] (64 deep rows + zeros),
    # wcb/wcc [128,5] for gram cells i in 0..7 / 8..15 (zeros where i>=j)
    wca_d = nc.dram_tensor("wca", [P, C], F32, kind="ExternalInput")
    wcb_d = nc.dram_tensor("wcb", [P, C], F32, kind="ExternalInput")
    wcc_d = nc.dram_tensor("wcc", [P, C], F32, kind="ExternalInput")
    bc_d = nc.dram_tensor("bc", [C], F32, kind="ExternalInput")
    out_d = nc.dram_tensor("out", [C, BL], F32, kind="ExternalOutput")

    with tile.TileContext(nc) as tc:
        with ExitStack() as ctx:
            _kernel_body(
                ctx, tc,
                xg_d, xl1_d, w1_d, w2_d, w3_d, b1_d, b2_d, b3_d,
                wca_d, wcb_d, wcc_d, bc_d, out_d,
            )
    nc.compile()
    return nc


def _kernel_body(ctx, tc, xg_d, xl1_d, w1_d, w2_d, w3_d, b1_d, b2_d, b3_d,
                 wca_d, wcb_d, wcc_d, bc_d, out_d):
    nc = tc.nc

    pool_const = ctx.enter_context(tc.tile_pool(name="const", bufs=1))
    pool_xt = ctx.enter_context(tc.tile_pool(name="xt", bufs=1))
    pool_w1 = ctx.enter_context(tc.tile_pool(name="w1", bufs=2))
    pool_h1 = ctx.enter_context(tc.tile_pool(name="h1", bufs=1))
    pool_h2 = ctx.enter_context(tc.tile_pool(name="h2", bufs=1))
    pool_gram = ctx.enter_context(tc.tile_pool(name="gram", bufs=3))
    pool_gramT = ctx.enter_context(tc.tile_pool(name="gramT", bufs=1))
    pool_psum = ctx.enter_context(tc.tile_pool(name="psum", bufs=2, space="PSUM"))
    pool_psum_g = ctx.enter_context(
        tc.tile_pool(name="psum_g", bufs=3, space="PSUM"))
    pool_dram = ctx.enter_context(tc.tile_pool(name="scratch", bufs=1, space="DRAM"))

    # ---- constants / weights ----
    b1_sb = pool_const.tile([P, OT1], F32)
    nc.scalar.dma_start(b1_sb[:], b1_d.ap().rearrange("(o p) -> p o", p=P))
    b2_sb = pool_const.tile([P, OT2], F32)
    nc.scalar.dma_start(b2_sb[:], b2_d.ap().rearrange("(o p) -> p o", p=P))
    b3_sb = pool_const.tile([HO, 1], F32)
    nc.scalar.dma_start(b3_sb[:], b3_d.ap().rearrange("(p o) -> p o", o=1))
    bc_sb = pool_const.tile([C, 1], F32)
    nc.scalar.dma_start(bc_sb[:], bc_d.ap().rearrange("(p o) -> p o", o=1))
    wca_sb = pool_const.tile([P, C], F32)
    nc.scalar.dma_start(wca_sb[:], wca_d.ap())
    wcb_sb = pool_const.tile([P, C], F32)
    nc.scalar.dma_start(wcb_sb[:], wcb_d.ap())
    wcc_sb = pool_const.tile([P, C], F32)
    nc.scalar.dma_start(wcc_sb[:], wcc_d.ap())

    # W2 slabs: [H1, 128] per output tile, resident
    w2_sb = pool_const.tile([P, OT2, K2, P], MLP_DT)
    for o2 in range(OT2):
        nc.scalar.dma_start(w2_sb[:, o2], w2_d.ap()[o2])
    # W3: [H2, 64] resident
    w3_sb = pool_const.tile([P, K3, HO], MLP_DT)
    nc.scalar.dma_start(w3_sb[:], w3_d.ap())

    # ---- x (both layouts) ----
    xg_sb = pool_xt.tile([P, 2, BL, N], BF16)
    for bc4 in range(2):
        for t in range(2):
            nc.sync.dma_start(
                xg_sb[:, t, bc4 * 256:(bc4 + 1) * 256],
                xg_d.ap()[:, t, bc4 * 256:(bc4 + 1) * 256],
            )
    xl1_sb = pool_xt.tile([P, K1, BL], BF16)
    for k4 in range(8):
        nc.sync.dma_start(
            xl1_sb[:, k4 * 4:(k4 + 1) * 4],
            xl1_d.ap()[:, k4 * 4:(k4 + 1) * 4],
        )

    def l1_rhs(kt):
        return xl1_sb[:, kt]

    # ---- layer-1 slab loader ----
    def load_w1_slab(ot):
        slab = pool_w1.tile([P, K1, P], MLP_DT, tag="w1slab")
        for cch in range(4):
            k0 = cch * 8
            nc.sync.dma_start(
                slab[:, k0:k0 + 8], w1_d.ap()[ot, :, k0:k0 + 8])
        return slab

    h1_sb = pool_h1.tile([P, K2, BL], MLP_DT)

    def l1_pass(ot, slab):
        ps = pool_psum.tile([P, BL], F32, tag="ps_mlp")
        for kt in range(K1):
            nc.tensor.matmul(
                ps[:],
                slab[:, kt],
                l1_rhs(kt),
                start=(kt == 0),
                stop=(kt == K1 - 1),
            )
        nc.scalar.activation(
            h1_sb[:, ot], ps[:], AFT.Relu, bias=b1_sb[:, ot:ot + 1])

    # ---- gram: 8 samples per 128x128 bf16 matmul ----
    def gram_operand(g, t):
        # columns m = s*16 + i are contiguous in the xg layout
        return xg_sb[:, t, g * 8:(g + 1) * 8, :]

    # scratch[p, g*128 + f] — partition-major so the scatter writes 4KB runs
    scratch = pool_dram.tile([P * NGROUP * P], F32)

    gram_chunks = []
    for ch in range(NCHUNK):
        gbuf = pool_gram.tile([P, GCHUNK, P], F32, tag="gbuf")
        for gl in range(GCHUNK):
            g = ch * GCHUNK + gl
            pg = pool_psum_g.tile([P, P], F32, tag="ps_gram")
            for t in range(2):
                nc.tensor.matmul(
                    pg[:],
                    gram_operand(g, t),
                    gram_operand(g, t),
                    start=(t == 0),
                    stop=(t == 1),
                )
            nc.vector.tensor_copy(gbuf[:, gl], pg[:])
        # scatter chunk to DRAM scratch: addr = p*(NGROUP*128) + g*128 + f
        nc.scalar.dma_start(
            bass.AP(scratch.tensor,
                    scratch.offset + ch * GCHUNK * P,
                    [[NGROUP * P, P], [1, GCHUNK * P]]),
            gbuf[:],
        )
        gram_chunks.append(gbuf)

    # gather diagonal 16x16 blocks into gramT halves [128, BL]
    # dst partition = (i%8)*16 + j, free = g*8 + s
    # src elem addr = g*16384 + s*2064 + i*128 + j  (within chunk)
    # gather diagonal 16x16 blocks into natural layout G_sb[b, (i,j)]:
    # one DMA per group, dims (s, i, j), b-contiguous on both sides
    g_sb = pool_gramT.tile([P, 4, N * N], F32, tag="g_nat", name="g_nat")
    RW = NGROUP * P  # scratch row width (per-partition elements)
    for g in range(NGROUP):
        # elem addr = (s*16+i)*RW + g*128 + s*16 + j
        src = bass.AP(
            scratch.tensor,
            scratch.offset + g * P,
            [[16 * RW + 16, 8], [RW, N], [1, N]],
        )
        dst = bass.AP(
            g_sb.tensor,
            g_sb.offset + (g % 16) * 8 * (4 * N * N) + (g // 16) * N * N,
            [[4 * N * N, 8], [N, N], [1, N]],
        )
        nc.gpsimd.dma_start(dst, src)

    # ---- layer 1 ----
    for ot in range(OT1):
        slab = load_w1_slab(ot)
        l1_pass(ot, slab)

    # PE-transpose 128x128 blocks of G_sb into gramT[h] = [cells, b]
    identity = pool_const.tile([P, P], F32)
    make_identity(nc, identity)
    gramT = [
        pool_gramT.tile([P, BL], F32, tag=f"gramT{h}", name=f"gramT{h}")
        for h in range(2)
    ]
    for bt in range(4):
        for h in range(2):
            pt = pool_psum_g.tile([P, P], F32, tag="ps_tr", name="ps_tr")
            nc.tensor.transpose(
                pt[:], g_sb[:, bt, h * P:(h + 1) * P], identity[:])
            nc.vector.tensor_copy(gramT[h][:, bt * P:(bt + 1) * P], pt[:])

    # ---- layer 2 ----
    h2_sb = pool_h2.tile([P, K3, BL], MLP_DT)
    for o2 in range(OT2):
        ps = pool_psum.tile([P, BL], F32, tag="ps_mlp")
        for kt in range(K2):
            nc.tensor.matmul(
                ps[:],
                w2_sb[:, o2, kt],
                h1_sb[:, kt],
                start=(kt == 0),
                stop=(kt == K2 - 1),
            )
        nc.scalar.activation(
            h2_sb[:, o2], ps[:], AFT.Relu, bias=b2_sb[:, o2:o2 + 1])

    # ---- layer 3 -> higher [64, BL] padded to 128 rows of zeros ----
    higher_sb = pool_gramT.tile([P, BL], F32, tag="higher")
    nc.gpsimd.memset(higher_sb[HO:, :], 0.0)
    ps3 = pool_psum.tile([P, BL], F32, tag="ps_mlp")
    for kt in range(K3):
        nc.tensor.matmul(
            ps3[:HO],
            w3_sb[:, kt],
            h2_sb[:, kt],
            start=(kt == 0),
            stop=(kt == K3 - 1),
        )
    nc.scalar.activation(
        higher_sb[:HO], ps3[:HO], AFT.Identity, bias=b3_sb[:])

    # ---- classifier: out[5, BL] = Wc.T @ [higher; gram cells] + bc ----
    psf = pool_psum.tile([P, BL], F32, tag="ps_mlp")
    nc.tensor.matmul(psf[:C], wca_sb[:], higher_sb[:], start=True, stop=False)
    nc.tensor.matmul(psf[:C], wcb_sb[:], gramT[0][:], start=False, stop=False)
    nc.tensor.matmul(psf[:C], wcc_sb[:], gramT[1][:], start=False, stop=True)
    out_sb = pool_const.tile([C, BL], F32)
    nc.scalar.activation(out_sb[:], psf[:C], AFT.Identity, bias=bc_sb[:])
    nc.sync.dma_start(out_d.ap(), out_sb[:])


_CACHED = None


def _get_program():
    global _CACHED
    if _CACHED is None:
        _CACHED = _build_program()
    return _CACHED


def prepare_in_maps(x, W1, b1, W2, b2, W3, b3, Wc, bc):
    x = np.ascontiguousarray(np.asarray(x, dtype=np.float32))
    W1 = np.asarray(W1, dtype=np.float32)
    W2 = np.asarray(W2, dtype=np.float32)
    W3 = np.ascontiguousarray(np.asarray(W3, dtype=np.float32))
    b1 = np.asarray(b1, dtype=np.float32)
    b2 = np.asarray(b2, dtype=np.float32)
    b3 = np.asarray(b3, dtype=np.float32)
    Wc = np.asarray(Wc, dtype=np.float32)
    bc = np.asarray(bc, dtype=np.float32)

    # host-side layout prep (replicated operands)
    import ml_dtypes
    wdt = ml_dtypes.bfloat16 if MLP_BF16 else np.float32
    # [ot, p, kt, o] / [ot2, p, kt, o] / [p, kt, o] partition-major
    w1r = np.ascontiguousarray(
        W1.reshape(K1, P, OT1, P).transpose(2, 1, 0, 3).astype(wdt))
    w2r = np.ascontiguousarray(
        W2.reshape(K2, P, OT2, P).transpose(2, 1, 0, 3).astype(wdt))
    W3 = np.ascontiguousarray(
        W3.reshape(K3, P, HO).transpose(1, 0, 2).astype(wdt))

    # classifier padding: fold triu-pair selection into gram-cell weights
    iu, ju = np.triu_indices(N, k=1)
    wc_gram = np.zeros((N * N, C), dtype=np.float32)
    wc_gram[iu * N + ju] = Wc[HO:]
    wca = np.zeros((P, C), dtype=np.float32)
    wca[:HO] = Wc[:HO]
    wcb = np.ascontiguousarray(wc_gram[:P])
    wcc = np.ascontiguousarray(wc_gram[P:])

    common = dict(
        w1r=w1r, w2r=w2r, w3=W3, b1=b1, b2=b2, b3=b3,
        wca=wca, wcb=wcb, wcc=wcc, bc=bc,
    )
    bf = ml_dtypes.bfloat16
    maps = []
    for c in range(NCORES):
        xs = x[c * BL:(c + 1) * BL].reshape(BL, N, 2, P).astype(bf)
        # xg[p, t, b, i];  xl1[p, kt, b] with kt = 2*i + t
        maps.append(dict(
            common,
            xg=np.ascontiguousarray(xs.transpose(3, 2, 0, 1)),
            xl1=np.ascontiguousarray(
                xs.transpose(3, 1, 2, 0).reshape(P, K1, BL)),
        ))
    return maps


def kernel(x, W1, b1, W2, b2, W3, b3, Wc, bc):
    in_maps = prepare_in_maps(x, W1, b1, W2, b2, W3, b3, Wc, bc)
    nc = _get_program()
    res = run_bass_kernel_spmd(nc, in_maps, core_ids=list(range(NCORES)))
    out = np.empty((B, C), dtype=np.float32)
    for c in range(NCORES):
        out[c * BL:(c + 1) * BL] = res.results[c]["out"].T
    return out
